# revision 2
# baseline (speedup 1.0000x reference)
"""Trainium2 Bass kernel for nn_CycleNet_EPD (ragged graph edge-phase decoder).

Math (per graph b, with La = edge_len[b], Ba = beta_len[b]):
  ef[e,:4]   = [x[src_e], x[dst_e]]                        (edge features)
  s[beta,:]  = sum_e |SCB[b,beta,e]| * ef[e,:]             (beta < Ba, e < La)
  emb        = relu(s@W1+b1)@W2+b2                         [Ba,64]
  A[beta,:]  = emb@W3a + b3                                [Ba,128]  (W3a=W3[:64])
  G[e,:]     = ef@W3b                                      [La,128]  (W3b=W3[64:])
  H[e,:]     = sum_{beta<Ba} relu(A[beta,:] + |SCB[b,beta,e]|*G[e,:])
               + (64-Ba)*relu(A_pad)          (A_pad = const row for padded beta)
  out[e,:]   = relu((H@W4 + 64*b4 + (64-Ba)*relu(A_pad)@W4)@... )
             = relu(h@W5+b5)@W6+b6 with h = H@W4 + vb
  rows with e >= La are zero.

Device mapping (per graph):
  - stage A: PE transposes |SCB| -> scb_T; small matmuls for s, emb, A, G, K0, vb
  - stage B per (beta, e-chunk): PE K=1 ones-matmul broadcasts |scb[beta, e-slice]|
    across 128 partitions into PSUM; DVE scalar_tensor_tensor computes
    |s2| * G_T (abs is a no-op re-guard); ACT applies relu with per-partition
    bias A_T[:,beta]; PE identity-matmul accumulates the beta-sum in PSUM.
  - out stage per e-chunk: three K=128 matmuls (W4, W5, W6) with ACT
    relu/bias epilogues, PE transpose to [e, 128], DMA to DRAM.

Sharding: per-core work items (graph, e0, e1) fill each core to ~total/8
La*Ba columns, splitting large graphs by edge range (stage A is recomputed on
each core touching a split graph; it is tiny). One NEFF; each core's exact
ragged schedule sits in its own branch of a partition-id If-tree.
Host does only data movement: gather of x rows by edge_index (edge feature
assembly), packing/padding per-core inputs, and scatter of per-core outputs
into the full [B*MAX_E, HID] result (padded rows stay zero).
"""

import sys

sys.path.insert(0, "/opt/trn_rl_repo")

import numpy as np

import concourse.bacc as bacc
import concourse.mybir as mybir
import concourse.tile as tile
from concourse import bass_utils

B, MAX_N, MAX_E, MAX_BETA = 16, 512, 1024, 64
NODE_F, HID = 2, 128
NCORES = 8
F32 = mybir.dt.float32
AF = mybir.ActivationFunctionType
ALU = mybir.AluOpType

ECHUNK = 512  # e-tile for stage B / out stage (one PSUM bank)


def _plan(edge_len, beta_len):
    """Per-core work items (g, e0, e1); large graphs split by edge range."""
    La = [max(1, min(MAX_E, int(v))) for v in edge_len]
    Ba = [max(1, min(MAX_BETA, int(v))) for v in beta_len]
    load = [La[b] * Ba[b] for b in range(B)]
    total = sum(load)
    target = -(-total // NCORES)
    order = sorted(range(B), key=lambda b: -load[b])
    cores = [[] for _ in range(NCORES)]
    c, used = 0, 0
    for g in order:
        e0 = 0
        while e0 < La[g]:
            cap = target - used
            if cap <= 0 and c < NCORES - 1:
                c, used = c + 1, 0
                cap = target
            ne = min(La[g] - e0, max(1, -(-cap // Ba[g])))
            if c == NCORES - 1:
                ne = La[g] - e0
            cores[c].append((g, e0, e0 + ne))
            used += ne * Ba[g]
            e0 += ne
    return La, Ba, cores


def kernel(x, SCB, edge_index, edge_len, beta_len,
           W1, b1, W2, b2, W3, b3, W4, b4, W5, b5, W6, b6):
    x = np.asarray(x, np.float32)
    SCB = np.asarray(SCB, np.float32)
    edge_index = np.asarray(edge_index, np.int32)
    La, Ba, cores = _plan(np.asarray(edge_len), np.asarray(beta_len))
    ngmax = max(len(c) for c in cores)

    # ---- host-side packing (data movement only) ----
    # edge features via index gather
    ef_all = []
    for b in range(B):
        src = edge_index[b, 0, : La[b]]
        dst = edge_index[b, 1, : La[b]]
        ef_all.append(np.concatenate([x[b][src], x[b][dst]], axis=1))  # [La,4]

    scb_off = [[0] * ngmax for _ in range(NCORES)]
    ef_off = [[0] * ngmax for _ in range(NCORES)]
    cmax = 1
    emax = 1
    for c in range(NCORES):
        co = 0
        eo = 0
        for i, (g, e0, e1) in enumerate(cores[c]):
            scb_off[c][i] = co
            ef_off[c][i] = eo
            co += (e1 - e0) * Ba[g]
            eo += La[g]
        cmax = max(cmax, co)
        emax = max(emax, eo)

    in_maps = []
    w_common = {
        "w1": np.ascontiguousarray(W1, np.float32),          # [4,64]
        "w2": np.ascontiguousarray(W2, np.float32),          # [64,64]
        "w3a": np.ascontiguousarray(W3[:64], np.float32),    # [64,128]
        "w3b": np.ascontiguousarray(W3[64:], np.float32),    # [4,128]
        "w4": np.ascontiguousarray(W4, np.float32),
        "w5": np.ascontiguousarray(W5, np.float32),
        "w6": np.ascontiguousarray(W6, np.float32),
        "b1c": np.ascontiguousarray(np.asarray(b1, np.float32)[:, None]),
        "b2c": np.ascontiguousarray(np.asarray(b2, np.float32)[:, None]),
        "b3c": np.ascontiguousarray(np.asarray(b3, np.float32)[:, None]),
        "b4x64": np.ascontiguousarray(64.0 * np.asarray(b4, np.float32)[:, None]),
        "b5c": np.ascontiguousarray(np.asarray(b5, np.float32)[:, None]),
        "b6c": np.ascontiguousarray(np.asarray(b6, np.float32)[:, None]),
        "ones": np.ones((1, 128), np.float32),
        "ident": np.eye(128, dtype=np.float32),
    }
    for c in range(NCORES):
        scb_pack = np.zeros((ngmax * 64, MAX_E), np.float32)
        scbcols = np.zeros((1, cmax), np.float32)
        eft = np.zeros((4, emax), np.float32)
        for i, (g, e0, e1) in enumerate(cores[c]):
            scb_pack[i * 64 : i * 64 + 64, : La[g]] = SCB[g][:, : La[g]]
            scbcols[0, scb_off[c][i] : scb_off[c][i] + (e1 - e0) * Ba[g]] = \
                np.abs(SCB[g][: Ba[g], e0:e1]).reshape(-1)
            eft[:, ef_off[c][i] : ef_off[c][i] + La[g]] = ef_all[g].T
        m = dict(w_common)
        m["scb_pack"] = scb_pack
        m["scbcols"] = scbcols
        m["eft"] = eft
        in_maps.append(m)

    # ---- build program ----
    nc = bacc.Bacc("TRN2", target_bir_lowering=False, debug=False,
                   num_devices=NCORES)
    d_in = {}
    for name, arr in in_maps[0].items():
        d_in[name] = nc.dram_tensor(name, list(arr.shape), F32,
                                    kind="ExternalInput")
    d_out = nc.dram_tensor("out", [ngmax * MAX_E, HID], F32,
                           kind="ExternalOutput")

    with tile.TileContext(nc) as tc:
        pid = nc.partition_id()
        with (
            tc.tile_pool(name="const", bufs=1) as cpool,
            tc.tile_pool(name="sbA", bufs=2) as sbA,
            tc.tile_pool(name="sbB", bufs=3) as sbB,
            tc.tile_pool(name="psS", bufs=2, space="PSUM") as psS,
            tc.tile_pool(name="psH", bufs=2, space="PSUM") as psH,
            tc.tile_pool(name="psO", bufs=2, space="PSUM") as psO,
            tc.tile_pool(name="psM", bufs=2, space="PSUM") as psM,
        ):
            cst = {}
            for name, shape in [
                ("w1", [4, 64]), ("w2", [64, 64]), ("w3a", [64, 128]),
                ("w3b", [4, 128]), ("w4", [128, 128]), ("w5", [128, 128]),
                ("w6", [128, 128]), ("b1c", [64, 1]), ("b2c", [64, 1]),
                ("b3c", [128, 1]), ("b4x64", [128, 1]), ("b5c", [128, 1]),
                ("b6c", [128, 1]), ("ones", [1, 128]), ("ident", [128, 128]),
            ]:
                t = cpool.tile(shape, F32, tag=name)
                nc.sync.dma_start(t[:], d_in[name].ap())
                cst[name] = t

            def build_graph(c, slot, g, es, ee):
                la, ba = La[g], Ba[g]
                lw = ee - es  # width of this core's edge slice
                nech = (la + 127) // 128  # 128-e chunks for transposes / s
                goff_s = scb_off[c][slot]
                goff_e = ef_off[c][slot]

                # ---- stage A ----
                scb_sb = sbA.tile([64, MAX_E], F32, tag="scb")
                nc.sync.dma_start(
                    scb_sb[:, :la],
                    d_in["scb_pack"].ap()[slot * 64 : slot * 64 + 64, :la],
                )
                # |SCB| on the rows used for s (in-place, ACT Abs)
                nc.scalar.activation(scb_sb[:ba, :la], scb_sb[:ba, :la],
                                     AF.Abs, bias=0.0, scale=1.0)
                eft_sb = sbA.tile([4, MAX_E], F32, tag="eft")
                nc.sync.dma_start(
                    eft_sb[:, :la], d_in["eft"].ap()[:, goff_e : goff_e + la]
                )
                # scb_T chunks [128e, 64b] via PE transpose
                scbT_sb = sbA.tile([128, 64 * 8], F32, tag="scbT")
                for ec in range(nech):
                    n = min(128, la - ec * 128)
                    tp = psM.tile([128, 64], F32, tag="pM")
                    nc.tensor.transpose(
                        tp[:n, :], scb_sb[:, ec * 128 : ec * 128 + n],
                        cst["ident"][:64, :64],
                    )
                    nc.scalar.copy(scbT_sb[:n, ec * 64 : ec * 64 + 64], tp[:n, :])
                # ef rows [e,4] per 128-chunk (for s); from eft via transpose
                efr_sb = sbA.tile([128, 4 * 8], F32, tag="efr")
                for ec in range(nech):
                    n = min(128, la - ec * 128)
                    tp2 = psM.tile([128, 4], F32, tag="pM")
                    nc.tensor.transpose(
                        tp2[:n, :], eft_sb[:, ec * 128 : ec * 128 + n],
                        cst["ident"][:4, :4],
                    )
                    nc.scalar.copy(efr_sb[:n, ec * 4 : ec * 4 + 4], tp2[:n, :])
                # G_T[h, e] = sum_k W3b[k,h] ef_T[k,e]
                GT_sb = sbA.tile([128, MAX_E], F32, tag="GT")
                for e0 in range(es, ee, ECHUNK):
                    n = min(ECHUNK, ee - e0)
                    pg = psM.tile([128, ECHUNK], F32, tag="pM")
                    nc.tensor.matmul(pg[:, :n], cst["w3b"][:],
                                     eft_sb[:, e0 : e0 + n],
                                     start=True, stop=True)
                    nc.scalar.copy(GT_sb[:, e0 : e0 + n], pg[:, :n])
                # s_T[k, beta] = sum_e ef[e,k] |scb|_T[e, beta]
                ps_s = psM.tile([4, 128], F32, tag="pM")
                for ec in range(nech):
                    n = min(128, la - ec * 128)
                    nc.tensor.matmul(
                        ps_s[:, :ba],
                        efr_sb[:n, ec * 4 : ec * 4 + 4],
                        scbT_sb[:n, ec * 64 : ec * 64 + ba],
                        start=(ec == 0), stop=(ec == nech - 1),
                    )
                s_sb = sbA.tile([4, 65], F32, tag="s")
                nc.vector.memset(s_sb[:], 0.0)
                nc.scalar.copy(s_sb[:, :ba], ps_s[:, :ba])
                # emb / A chain (one padded col at index ba -> A_pad)
                nb = ba + 1
                pe1 = psM.tile([64, 65], F32, tag="pM")
                nc.tensor.matmul(pe1[:, :nb], cst["w1"][:], s_sb[:, :nb],
                                 start=True, stop=True)
                e1_sb = sbA.tile([64, 65], F32, tag="e1")
                nc.scalar.activation(e1_sb[:, :nb], pe1[:, :nb], AF.Relu,
                                     bias=cst["b1c"][:], scale=1.0)
                pe2 = psM.tile([64, 65], F32, tag="pM")
                nc.tensor.matmul(pe2[:, :nb], cst["w2"][:], e1_sb[:, :nb],
                                 start=True, stop=True)
                e2_sb = sbA.tile([64, 65], F32, tag="e2")
                nc.scalar.activation(e2_sb[:, :nb], pe2[:, :nb], AF.Identity,
                                     bias=cst["b2c"][:], scale=1.0)
                pa = psM.tile([128, 65], F32, tag="pM")
                nc.tensor.matmul(pa[:, :nb], cst["w3a"][:], e2_sb[:, :nb],
                                 start=True, stop=True)
                A_sb = sbA.tile([128, 65], F32, tag="A")
                nc.scalar.activation(A_sb[:, :nb], pa[:, :nb], AF.Identity,
                                     bias=cst["b3c"][:], scale=1.0)
                # K0 = relu(A_pad); vb = (64-Ba) * K0@W4 + 64*b4
                K0_sb = sbA.tile([128, 1], F32, tag="K0")
                nc.scalar.activation(K0_sb[:], A_sb[:, ba : ba + 1], AF.Relu,
                                     bias=0.0, scale=1.0)
                pk = psM.tile([128, 1], F32, tag="pM")
                nc.tensor.matmul(pk[:], cst["w4"][:], K0_sb[:],
                                 start=True, stop=True)
                vb_sb = sbA.tile([128, 1], F32, tag="vb")
                nc.scalar.activation(vb_sb[:], pk[:], AF.Identity,
                                     bias=cst["b4x64"][:],
                                     scale=float(64 - ba))

                # ---- stage B + out stage, per 512-e chunk ----
                for e0 in range(es, ee, ECHUNK):
                    n = min(ECHUNK, ee - e0)
                    pH = psH.tile([128, ECHUNK], F32, tag="H")
                    for bi in range(ba):
                        co = goff_s + bi * lw + (e0 - es)
                        srow = sbB.tile([1, ECHUNK], F32, tag="srow")
                        nc.sync.dma_start(
                            srow[:, :n], d_in["scbcols"].ap()[:, co : co + n]
                        )
                        ps2 = psS.tile([128, ECHUNK], F32, tag="s2")
                        nc.tensor.matmul(ps2[:, :n], cst["ones"][:],
                                         srow[:, :n], start=True, stop=True)
                        t_sb = sbB.tile([128, ECHUNK], F32, tag="t")
                        nc.vector.tensor_mul(
                            t_sb[:, :n], ps2[:, :n], GT_sb[:, e0 : e0 + n]
                        )
                        r_sb = sbB.tile([128, ECHUNK], F32, tag="r")
                        nc.scalar.activation(r_sb[:, :n], t_sb[:, :n], AF.Relu,
                                             bias=A_sb[:, bi : bi + 1],
                                             scale=1.0)
                        nc.tensor.matmul(pH[:, :n], cst["ident"][:],
                                         r_sb[:, :n], start=(bi == 0),
                                         stop=(bi == ba - 1))
                    H_sb = sbB.tile([128, ECHUNK], F32, tag="Hs")
                    nc.scalar.copy(H_sb[:, :n], pH[:, :n])
                    p1 = psO.tile([128, ECHUNK], F32, tag="pO")
                    nc.tensor.matmul(p1[:, :n], cst["w4"][:], H_sb[:, :n],
                                     start=True, stop=True)
                    r5 = sbB.tile([128, ECHUNK], F32, tag="r5")
                    nc.scalar.activation(r5[:, :n], p1[:, :n], AF.Identity,
                                         bias=vb_sb[:], scale=1.0)
                    p2 = psO.tile([128, ECHUNK], F32, tag="pO")
                    nc.tensor.matmul(p2[:, :n], cst["w5"][:], r5[:, :n],
                                     start=True, stop=True)
                    r6 = sbB.tile([128, ECHUNK], F32, tag="r6")
                    nc.scalar.activation(r6[:, :n], p2[:, :n], AF.Relu,
                                         bias=cst["b5c"][:], scale=1.0)
                    p3 = psO.tile([128, ECHUNK], F32, tag="pO")
                    nc.tensor.matmul(p3[:, :n], cst["w6"][:], r6[:, :n],
                                     start=True, stop=True)
                    o_sb = sbB.tile([128, ECHUNK], F32, tag="o")
                    nc.scalar.activation(o_sb[:, :n], p3[:, :n], AF.Identity,
                                         bias=cst["b6c"][:], scale=1.0)
                    for et in range(0, n, 128):
                        m = min(128, n - et)
                        po = psM.tile([128, 128], F32, tag="pM")
                        nc.tensor.transpose(po[:m, :], o_sb[:, et : et + m],
                                            cst["ident"][:])
                        oT = sbB.tile([128, 128], F32, tag="oT")
                        nc.scalar.copy(oT[:m, :], po[:m, :])
                        r0 = slot * MAX_E + e0 + et
                        nc.sync.dma_start(d_out.ap()[r0 : r0 + m, :],
                                          oT[:m, :])

            def build_core(c):
                for slot, (g, e0, e1) in enumerate(cores[c]):
                    build_graph(c, slot, g, e0, e1)

            def dispatch(lo, hi):
                if hi - lo == 1:
                    build_core(lo)
                    return
                mid = (lo + hi) // 2
                with tc.If(pid < mid) as cmp:
                    dispatch(lo, mid)
                with cmp.Else():
                    dispatch(mid, hi)

            dispatch(0, NCORES)

    import os
    if os.environ.get("KERNEL_BUILD_ONLY"):
        return np.zeros((B * MAX_E, HID), np.float32)
    nc.compile()
    if os.environ.get("KERNEL_COMPILE_ONLY"):
        import tempfile
        neff = bass_utils.compile_bass_kernel(nc, tempfile.mkdtemp())
        print("NEFF:", neff)
        return np.zeros((B * MAX_E, HID), np.float32)
    trace = bool(os.environ.get("KERNEL_TRACE"))
    res = bass_utils.run_bass_kernel_spmd(
        nc, in_maps, core_ids=list(range(NCORES)),
        trace=trace,
        trace_cores=list(range(NCORES)) if trace else None,
    )
    global LAST_EXEC_NS, LAST_RESULTS
    LAST_RESULTS = res
    LAST_EXEC_NS = res.exec_time_ns

    out = np.zeros((B * MAX_E, HID), np.float32)
    for c in range(NCORES):
        oc = res.results[c]["out"]
        for slot, (g, e0, e1) in enumerate(cores[c]):
            out[g * MAX_E + e0 : g * MAX_E + e1] = \
                oc[slot * MAX_E + e0 : slot * MAX_E + e1]
    return out



# revision 11
# speedup vs baseline: 1.7861x; 1.7861x over previous
"""Trainium2 Bass kernel for nn_CycleNet_EPD (ragged graph edge-phase decoder).

Math (per graph b, with La = edge_len[b], Ba = beta_len[b]):
  ef[e,:4]   = [x[src_e], x[dst_e]]                        (edge features)
  s[beta,:]  = sum_e |SCB[b,beta,e]| * ef[e,:]             (beta < Ba, e < La)
  emb        = relu(s@W1+b1)@W2+b2                         [Ba,64]
  A[beta,:]  = emb@W3a + b3                                [Ba,128]  (W3a=W3[:64])
  G[e,:]     = ef@W3b                                      [La,128]  (W3b=W3[64:])
  H[e,:]     = sum_{beta<Ba} relu(A[beta,:] + |SCB[b,beta,e]|*G[e,:])
  out[e,:]   = relu((H@W4 + vb)@W5+b5)@W6+b6
               vb = 64*b4 + (64-Ba)*relu(A_pad)@W4  (A_pad: padded-beta row)
  rows with e >= La are zero.

Device mapping (per graph), [h, e] layout, e-chunks of 512:
  - G is rank 4 (G = W3b^T ef), so scb_beta (x) G = W3b^T (ef (x) scb_beta).
    Per 32-beta group one PE "expand" matmul broadcasts scb rows to 4x32
    partitions and one DVE multiply builds EFS[4i+k,e] = ef[k,e]*|scb|[g0+i,e]
    (bf16).  Per beta, a single K=128 matmul with masked stacked weights
    w3bm (rows 4i..4i+3 = W3b, zeros elsewhere) yields scb_beta*G in PSUM.
  - relu(+A bias) on ACT (majority) / DVE tensor_scalar (minority, balance).
  - The beta-sum is folded into W4: r@W4b accumulates into one PSUM bank
    (start/stop over the chunk's betas); a minority of betas accumulate r on
    DVE into H2 which is flushed through W4b with one extra matmul.
  - out stage: W5/W6 bf16 matmuls with ACT relu/bias epilogues, PE transpose
    to [e, 128], DMA to DRAM.
  - bf16 (1 cyc/row) matmuls in the hot loop; amortized fp32 elsewhere.

Sharding: per-core work items (graph, e0, e1) fill each core to ~total/8
La*Ba columns, splitting large graphs by edge range (stage A is recomputed on
each core touching a split graph; it is tiny). One NEFF; each core's exact
ragged schedule sits in its own branch of a partition-id If-tree.
Host does only data movement: gather of x rows by edge_index (edge feature
assembly), packing/padding per-core inputs, and scatter of per-core outputs
into the full [B*MAX_E, HID] result (padded rows stay zero).
"""

import sys

sys.path.insert(0, "/opt/trn_rl_repo")

import ml_dtypes
import numpy as np

import concourse.bacc as bacc
import concourse.mybir as mybir
import concourse.tile as tile
from concourse import bass_utils

B, MAX_N, MAX_E, MAX_BETA = 16, 512, 1024, 64
NODE_F, HID = 2, 128
NCORES = 8
F32 = mybir.dt.float32
F32R = mybir.dt.float32r
BF16 = mybir.dt.bfloat16
AF = mybir.ActivationFunctionType
ALU = mybir.AluOpType
NPBF16 = ml_dtypes.bfloat16

ECHUNK = 512  # e-tile for stage B / out stage (one PSUM bank)


def _r(ap):
    """View an fp32 AP as float32r for 1-cycle/row PE matmuls."""
    return ap.bitcast(F32R)


def _relu_on_dve(b):
    return b % 5 == 2


def _acc_on_dve(b):
    return b % 5 in (1, 3)


def _plan(edge_len, beta_len):
    """Per-core work items (g, e0, e1); large graphs split by edge range."""
    La = [max(1, min(MAX_E, int(v))) for v in edge_len]
    Ba = [max(1, min(MAX_BETA, int(v))) for v in beta_len]
    load = [La[b] * Ba[b] for b in range(B)]
    total = sum(load)
    target = -(-total // NCORES)
    order = sorted(range(B), key=lambda b: -load[b])
    cores = [[] for _ in range(NCORES)]
    c, used = 0, 0
    for g in order:
        e0 = 0
        while e0 < La[g]:
            cap = target - used
            if cap <= 0 and c < NCORES - 1:
                c, used = c + 1, 0
                cap = target
            ne = min(La[g] - e0, max(1, -(-cap // Ba[g])))
            if c == NCORES - 1:
                ne = La[g] - e0
            cores[c].append((g, e0, e0 + ne))
            used += ne * Ba[g]
            e0 += ne
    return La, Ba, cores


def kernel(x, SCB, edge_index, edge_len, beta_len,
           W1, b1, W2, b2, W3, b3, W4, b4, W5, b5, W6, b6):
    x = np.asarray(x, np.float32)
    SCB = np.asarray(SCB, np.float32)
    edge_index = np.asarray(edge_index, np.int32)
    La, Ba, cores = _plan(np.asarray(edge_len), np.asarray(beta_len))
    ngmax = max(len(c) for c in cores)

    # ---- host-side packing (data movement only) ----
    # edge features via index gather
    ef_all = []
    for b in range(B):
        src = edge_index[b, 0, : La[b]]
        dst = edge_index[b, 1, : La[b]]
        ef_all.append(np.concatenate([x[b][src], x[b][dst]], axis=1))  # [La,4]

    ef_off = [[0] * ngmax for _ in range(NCORES)]
    emax = 1
    for c in range(NCORES):
        eo = 0
        for i, (g, e0, e1) in enumerate(cores[c]):
            ef_off[c][i] = eo
            eo += La[g]
        emax = max(emax, eo)

    # constants for the rank-4 stage-B restructure
    W3b = np.ascontiguousarray(W3[64:], np.float32)       # [4,128]
    exp64 = np.zeros((64, 128), np.float32)               # expand 32b -> 4x32p
    for i in range(64):
        exp64[i, 4 * (i % 32) : 4 * (i % 32) + 4] = 1.0
    tile32 = np.zeros((4, 128), np.float32)               # tile ef 32x
    for i in range(32):
        for k in range(4):
            tile32[k, 4 * i + k] = 1.0
    w3bm = np.zeros((128, 32 * 128), np.float32)          # masked stacked W3b
    for i in range(32):
        w3bm[4 * i : 4 * i + 4, i * 128 : (i + 1) * 128] = W3b

    in_maps = []
    w_common = {
        "w1": np.ascontiguousarray(W1, np.float32),          # [4,64]
        "w2": np.ascontiguousarray(W2, np.float32),          # [64,64]
        "w3a": np.ascontiguousarray(W3[:64], np.float32),    # [64,128]
        "w4": np.ascontiguousarray(W4, np.float32),
        "exp64": exp64,
        "tile32": tile32,
        "w3bm": w3bm.astype(NPBF16),
        "w4b": np.ascontiguousarray(W4, np.float32).astype(NPBF16),
        "w5b": np.ascontiguousarray(W5, np.float32).astype(NPBF16),
        "w6b": np.ascontiguousarray(W6, np.float32).astype(NPBF16),
        "b1c": np.ascontiguousarray(np.asarray(b1, np.float32)[:, None]),
        "b2c": np.ascontiguousarray(np.asarray(b2, np.float32)[:, None]),
        "b3c": np.ascontiguousarray(np.asarray(b3, np.float32)[:, None]),
        "b4x64": np.ascontiguousarray(64.0 * np.asarray(b4, np.float32)[:, None]),
        "b5c": np.ascontiguousarray(np.asarray(b5, np.float32)[:, None]),
        "b6c": np.ascontiguousarray(np.asarray(b6, np.float32)[:, None]),
        "ident": np.eye(128, dtype=np.float32),
    }
    for c in range(NCORES):
        scb_pack = np.zeros((ngmax * 64, MAX_E), np.float32)
        eft = np.zeros((4, emax), np.float32)
        for i, (g, e0, e1) in enumerate(cores[c]):
            scb_pack[i * 64 : i * 64 + 64, : La[g]] = SCB[g][:, : La[g]]
            eft[:, ef_off[c][i] : ef_off[c][i] + La[g]] = ef_all[g].T
        m = dict(w_common)
        m["scb_pack"] = scb_pack
        m["eft"] = eft
        in_maps.append(m)

    # ---- build program ----
    nc = bacc.Bacc("TRN2", target_bir_lowering=False, debug=False,
                   num_devices=NCORES)
    d_in = {}
    for name, arr in in_maps[0].items():
        dt = BF16 if arr.dtype == NPBF16 else F32
        d_in[name] = nc.dram_tensor(name, list(arr.shape), dt,
                                    kind="ExternalInput")
    d_out = nc.dram_tensor("out", [ngmax * MAX_E, HID], F32,
                           kind="ExternalOutput")

    with tile.TileContext(nc) as tc:
        pid = nc.partition_id()
        with (
            tc.tile_pool(name="const", bufs=1) as cpool,
            tc.tile_pool(name="sbA", bufs=2) as sbA,
            tc.tile_pool(name="sbB", bufs=3) as sbB,
            tc.tile_pool(name="psS", bufs=2, space="PSUM") as psS,
            tc.tile_pool(name="psG", bufs=2, space="PSUM") as psG,
            tc.tile_pool(name="psH", bufs=2, space="PSUM") as psH,
            tc.tile_pool(name="psO", bufs=1, space="PSUM") as psO,
        ):
            cst = {}
            for name, shape, dt in [
                ("w1", [4, 64], F32), ("w2", [64, 64], F32),
                ("w3a", [64, 128], F32), ("w4", [128, 128], F32),
                ("exp64", [64, 128], F32), ("tile32", [4, 128], F32),
                ("w3bm", [128, 32 * 128], BF16),
                ("w4b", [128, 128], BF16), ("w5b", [128, 128], BF16),
                ("w6b", [128, 128], BF16),
                ("b1c", [64, 1], F32), ("b2c", [64, 1], F32),
                ("b3c", [128, 1], F32), ("b4x64", [128, 1], F32),
                ("b5c", [128, 1], F32), ("b6c", [128, 1], F32),
                ("ident", [128, 128], F32),
            ]:
                t = cpool.tile(shape, dt, tag=name)
                nc.sync.dma_start(t[:], d_in[name].ap())
                cst[name] = t

            def build_graph(c, slot, g, es, ee):
                la, ba = La[g], Ba[g]
                nech = (la + 127) // 128  # 128-e chunks for transposes / s
                goff_e = ef_off[c][slot]

                # ---- stage A ----
                scb_sb = sbA.tile([64, MAX_E], F32, tag="scb")
                nc.sync.dma_start(
                    scb_sb[:, :la],
                    d_in["scb_pack"].ap()[slot * 64 : slot * 64 + 64, :la],
                )
                # |SCB| in-place (ACT Abs); rows >= ba never read
                nc.scalar.activation(scb_sb[:ba, :la], scb_sb[:ba, :la],
                                     AF.Abs, bias=0.0, scale=1.0)
                eft_sb = sbA.tile([4, MAX_E], F32, tag="eft")
                nc.sync.dma_start(
                    eft_sb[:, :la], d_in["eft"].ap()[:, goff_e : goff_e + la]
                )
                # scb_T chunks [128e, 64b] via PE transpose (for s)
                scbT_sb = sbA.tile([128, 64 * 8], F32, tag="scbT")
                for ec in range(nech):
                    n = min(128, la - ec * 128)
                    tp = psO.tile([128, 64], F32, tag="pM")
                    nc.tensor.transpose(
                        tp[:n, :], scb_sb[:, ec * 128 : ec * 128 + n],
                        cst["ident"][:64, :64],
                    )
                    nc.vector.tensor_copy(scbT_sb[:n, ec * 64 : ec * 64 + 64],
                                          tp[:n, :])
                # ef rows [e,4] per 128-chunk (for s); from eft via transpose
                efr_sb = sbA.tile([128, 4 * 8], F32, tag="efr")
                for ec in range(nech):
                    n = min(128, la - ec * 128)
                    tp2 = psO.tile([128, 4], F32, tag="pM")
                    nc.tensor.transpose(
                        tp2[:n, :], eft_sb[:, ec * 128 : ec * 128 + n],
                        cst["ident"][:4, :4],
                    )
                    nc.vector.tensor_copy(efr_sb[:n, ec * 4 : ec * 4 + 4],
                                          tp2[:n, :])
                # s_T[k, beta] = sum_e ef[e,k] |scb|_T[e, beta]
                ps_s = psO.tile([4, 128], F32, tag="pM")
                for ec in range(nech):
                    n = min(128, la - ec * 128)
                    nc.tensor.matmul(
                        ps_s[:, :ba],
                        efr_sb[:n, ec * 4 : ec * 4 + 4],
                        scbT_sb[:n, ec * 64 : ec * 64 + ba],
                        start=(ec == 0), stop=(ec == nech - 1),
                    )
                s_sb = sbA.tile([4, 65], F32, tag="s")
                nc.vector.memset(s_sb[:], 0.0)
                nc.vector.tensor_copy(s_sb[:, :ba], ps_s[:, :ba])
                # emb / A chain (one padded col at index ba -> A_pad)
                nb = ba + 1
                pe1 = psO.tile([64, 65], F32, tag="pM")
                nc.tensor.matmul(pe1[:, :nb], cst["w1"][:], s_sb[:, :nb],
                                 start=True, stop=True)
                e1_sb = sbA.tile([64, 65], F32, tag="e1")
                nc.scalar.activation(e1_sb[:, :nb], pe1[:, :nb], AF.Relu,
                                     bias=cst["b1c"][:], scale=1.0)
                pe2 = psO.tile([64, 65], F32, tag="pM")
                nc.tensor.matmul(pe2[:, :nb], cst["w2"][:], e1_sb[:, :nb],
                                 start=True, stop=True)
                e2_sb = sbA.tile([64, 65], F32, tag="e2")
                nc.scalar.activation(e2_sb[:, :nb], pe2[:, :nb], AF.Identity,
                                     bias=cst["b2c"][:], scale=1.0)
                pa = psO.tile([128, 65], F32, tag="pM")
                nc.tensor.matmul(pa[:, :nb], cst["w3a"][:], e2_sb[:, :nb],
                                 start=True, stop=True)
                A_sb = sbA.tile([128, 65], F32, tag="A")
                nc.scalar.activation(A_sb[:, :nb], pa[:, :nb], AF.Identity,
                                     bias=cst["b3c"][:], scale=1.0)
                # K0 = relu(A_pad); vb = (64-Ba) * K0@W4 + 64*b4
                K0_sb = sbA.tile([128, 1], F32, tag="K0")
                nc.scalar.activation(K0_sb[:], A_sb[:, ba : ba + 1], AF.Relu,
                                     bias=0.0, scale=1.0)
                pk = psO.tile([128, 1], F32, tag="pM")
                nc.tensor.matmul(pk[:], cst["w4"][:], K0_sb[:],
                                 start=True, stop=True)
                vb_sb = sbA.tile([128, 1], F32, tag="vb")
                nc.scalar.activation(vb_sb[:], pk[:], AF.Identity,
                                     bias=cst["b4x64"][:],
                                     scale=float(64 - ba))

                # ---- stage B + out stage, per 512-e chunk ----
                pe_acc = [b for b in range(ba) if not _acc_on_dve(b)]
                dve_acc = [b for b in range(ba) if _acc_on_dve(b)]
                if not pe_acc:  # guarantee pW4 gets a start
                    pe_acc = [dve_acc.pop(0)]
                for e0 in range(es, ee, ECHUNK):
                    n = min(ECHUNK, ee - e0)
                    # EFS feature tile: ef tiled 32x along partitions, bf16
                    peft = psG.tile([128, ECHUNK], F32, tag="pG")
                    nc.tensor.matmul(peft[:, :n], cst["tile32"][:],
                                     eft_sb[:, e0 : e0 + n],
                                     start=True, stop=True)
                    eftb = sbB.tile([128, ECHUNK], BF16, tag="eftb")
                    nc.vector.tensor_copy(eftb[:, :n], peft[:, :n])

                    pW4 = psH.tile([128, ECHUNK], F32, tag="H")
                    H2 = sbB.tile([128, ECHUNK], F32, tag="H2")
                    h2_open = False
                    for g0 in range(0, ba, 32):
                        gsz = min(32, ba - g0)
                        P1 = psS.tile([128, ECHUNK], F32, tag="P1")
                        nc.tensor.matmul(
                            P1[:, :n],
                            cst["exp64"][g0 : g0 + gsz, :],
                            scb_sb[g0 : g0 + gsz, e0 : e0 + n],
                            start=True, stop=True)
                        efs = sbB.tile([128, ECHUNK], BF16, tag="efs")
                        nc.vector.tensor_mul(efs[:, :n], P1[:, :n],
                                             eftb[:, :n])
                        for i in range(gsz):
                            b = g0 + i
                            pG = psG.tile([128, ECHUNK], F32, tag="pG")
                            nc.tensor.matmul(
                                pG[:, :n],
                                cst["w3bm"][:, i * 128 : (i + 1) * 128],
                                efs[:, :n], start=True, stop=True)
                            r = sbB.tile([128, ECHUNK], BF16, tag="r")
                            if _relu_on_dve(b):
                                nc.vector.tensor_scalar(
                                    r[:, :n], pG[:, :n],
                                    A_sb[:, b : b + 1], 0.0,
                                    ALU.add, ALU.max)
                            else:
                                nc.scalar.activation(
                                    r[:, :n], pG[:, :n], AF.Relu,
                                    bias=A_sb[:, b : b + 1], scale=1.0)
                            if b in pe_acc:
                                nc.tensor.matmul(
                                    pW4[:, :n], cst["w4b"][:], r[:, :n],
                                    start=(b == pe_acc[0]),
                                    stop=(b == pe_acc[-1] and not dve_acc))
                            elif not h2_open:
                                nc.vector.tensor_copy(H2[:, :n], r[:, :n])
                                h2_open = True
                            else:
                                nc.vector.tensor_add(H2[:, :n], H2[:, :n],
                                                     r[:, :n])
                    if dve_acc:
                        H2b = sbB.tile([128, ECHUNK], BF16, tag="H2b")
                        nc.vector.tensor_copy(H2b[:, :n], H2[:, :n])
                        nc.tensor.matmul(pW4[:, :n], cst["w4b"][:],
                                         H2b[:, :n], start=False, stop=True)
                    # out stage: h = pW4 + vb; relu(h@W5+b5)@W6+b6
                    r5 = sbB.tile([128, ECHUNK], BF16, tag="r5")
                    nc.scalar.activation(r5[:, :n], pW4[:, :n], AF.Identity,
                                         bias=vb_sb[:], scale=1.0)
                    p2 = psO.tile([128, ECHUNK], F32, tag="pO")
                    nc.tensor.matmul(p2[:, :n], cst["w5b"][:], r5[:, :n],
                                     start=True, stop=True)
                    r6 = sbB.tile([128, ECHUNK], BF16, tag="r6")
                    nc.scalar.activation(r6[:, :n], p2[:, :n], AF.Relu,
                                         bias=cst["b5c"][:], scale=1.0)
                    p3 = psO.tile([128, ECHUNK], F32, tag="pO")
                    nc.tensor.matmul(p3[:, :n], cst["w6b"][:], r6[:, :n],
                                     start=True, stop=True)
                    o_sb = sbB.tile([128, ECHUNK], F32, tag="o")
                    nc.scalar.activation(o_sb[:, :n], p3[:, :n], AF.Identity,
                                         bias=cst["b6c"][:], scale=1.0)
                    for et in range(0, n, 128):
                        m = min(128, n - et)
                        po = psO.tile([128, 128], F32, tag="pM")
                        nc.tensor.transpose(po[:m, :], o_sb[:, et : et + m],
                                            cst["ident"][:])
                        oT = sbB.tile([128, 128], F32, tag="oT")
                        nc.scalar.copy(oT[:m, :], po[:m, :])
                        r0 = slot * MAX_E + e0 + et
                        nc.sync.dma_start(d_out.ap()[r0 : r0 + m, :],
                                          oT[:m, :])

            def build_core(c):
                for slot, (g, e0, e1) in enumerate(cores[c]):
                    build_graph(c, slot, g, e0, e1)

            def dispatch(lo, hi):
                if hi - lo == 1:
                    build_core(lo)
                    return
                mid = (lo + hi) // 2
                with tc.If(pid < mid) as cmp:
                    dispatch(lo, mid)
                with cmp.Else():
                    dispatch(mid, hi)

            dispatch(0, NCORES)

    import os
    if os.environ.get("KERNEL_BUILD_ONLY"):
        return np.zeros((B * MAX_E, HID), np.float32)
    nc.compile()
    if os.environ.get("KERNEL_COMPILE_ONLY"):
        import tempfile
        neff = bass_utils.compile_bass_kernel(nc, tempfile.mkdtemp())
        print("NEFF:", neff)
        return np.zeros((B * MAX_E, HID), np.float32)
    trace = bool(os.environ.get("KERNEL_TRACE"))
    res = bass_utils.run_bass_kernel_spmd(
        nc, in_maps, core_ids=list(range(NCORES)),
        trace=trace,
        trace_cores=list(range(NCORES)) if trace else None,
    )
    global LAST_EXEC_NS, LAST_RESULTS
    LAST_RESULTS = res
    LAST_EXEC_NS = res.exec_time_ns

    out = np.zeros((B * MAX_E, HID), np.float32)
    for c in range(NCORES):
        oc = res.results[c]["out"]
        for slot, (g, e0, e1) in enumerate(cores[c]):
            out[g * MAX_E + e0 : g * MAX_E + e1] = \
                oc[slot * MAX_E + e0 : slot * MAX_E + e1]
    return out


# revision 14
# speedup vs baseline: 2.8596x; 1.6010x over previous
"""Trainium2 Bass kernel for nn_CycleNet_EPD (ragged graph edge-phase decoder).

Math (per graph b, with La = edge_len[b], Ba = beta_len[b]):
  ef[e,:4]   = [x[src_e], x[dst_e]]                        (edge features)
  s[beta,:]  = sum_e |SCB[b,beta,e]| * ef[e,:]             (beta < Ba, e < La)
  emb        = relu(s@W1+b1)@W2+b2                         [Ba,64]
  A[beta,:]  = emb@W3a + b3                                [Ba,128]  (W3a=W3[:64])
  G[e,:]     = ef@W3b                                      [La,128]  (W3b=W3[64:])
  H[e,:]     = sum_{beta<Ba} relu(A[beta,:] + |SCB[b,beta,e]|*G[e,:])
  out[e,:]   = relu((H@W4 + vb)@W5+b5)@W6+b6
               vb = 64*b4 + (64-Ba)*relu(A_pad)@W4  (A_pad: padded-beta row)
  rows with e >= La are zero.

Device mapping (per graph), [h, e] layout, e-chunks of 512:
  - G is rank 4 (G = W3b^T ef), so scb_beta (x) G = W3b^T (ef (x) scb_beta).
    Per 32-beta group one PE "expand" matmul (bf16) broadcasts scb rows to
    4x32 partitions; one DVE multiply with the host-shipped 32x-tiled edge
    features (eft32) builds EFS[4i+k,e] = ef[k,e]*|scb|[g0+i,e] in bf16.
    Per beta, a single K=128 matmul with masked stacked weights w3bm
    (rows 4i..4i+3 = W3b, zeros elsewhere) yields scb_beta*G in PSUM.
  - relu(+A bias) on ACT (majority) / DVE tensor_scalar (minority, balance).
  - The beta-sum is folded into W4: r@W4b accumulates into one PSUM bank
    (start/stop over the chunk's betas); a minority of betas accumulate r on
    DVE into H2 which is flushed through W4b with one extra matmul.
  - out stage: W5/W6 bf16 matmuls with ACT relu/bias epilogues, PE transpose
    to [e, 128], one batched DMA per chunk to DRAM.
  - all hot-loop matmuls bf16 (1 cyc/row); fp32 only in the tiny emb chain.

Sharding: per-core work items (graph, e0, e1); effective load model counts
columns Ba*ne plus per-slice and per-edge-column fixed overheads so cores
with many small graphs are not overloaded. One NEFF; each core's ragged
schedule sits in its own branch of a partition-id If-tree.
Host does only data movement: gather of x rows by edge_index, packing /
replication / dtype casts of inputs, and scatter of per-core outputs into
the full [B*MAX_E, HID] result (padded rows stay zero).
"""

import sys

sys.path.insert(0, "/opt/trn_rl_repo")

import ml_dtypes
import numpy as np

import concourse.bacc as bacc
import concourse.mybir as mybir
import concourse.tile as tile
from concourse import bass_utils

B, MAX_N, MAX_E, MAX_BETA = 16, 512, 1024, 64
NODE_F, HID = 2, 128
NCORES = 8
F32 = mybir.dt.float32
BF16 = mybir.dt.bfloat16
AF = mybir.ActivationFunctionType
ALU = mybir.AluOpType
NPBF16 = ml_dtypes.bfloat16

ECHUNK = 512   # e-tile for stage B / out stage (one PSUM bank)
GFIX = 3000    # planner: per-slice fixed cost (stage A), in column units
OUT_W = 5      # planner: out-stage cost per edge column, in column units


def _relu_on_dve(b):
    return b % 5 == 2


def _acc_on_dve(b):
    return b % 5 in (1, 3)


def _plan(edge_len, beta_len):
    """Per-core work items (g, e0, e1); large graphs split by edge range.

    Effective load = (Ba + OUT_W) * ne + GFIX per slice, balancing stage-B
    columns plus out-stage and per-graph fixed overheads.  Split oversized
    graphs, LPT-assign pieces, then iteratively shave edges from the max
    core onto the min core."""
    La = [max(1, min(MAX_E, int(v))) for v in edge_len]
    Ba = [max(1, min(MAX_BETA, int(v))) for v in beta_len]

    def el(g, ne):
        return (Ba[g] + OUT_W) * ne + GFIX

    total = sum(el(g, La[g]) for g in range(B))
    target = total / NCORES
    pieces = []
    for g in range(B):
        k = max(1, min(round(el(g, La[g]) / target + 0.25), -(-La[g] // 64)))
        base, rem = divmod(La[g], k)
        e0 = 0
        for j in range(k):
            ne = base + (1 if j < rem else 0)
            pieces.append((g, e0, e0 + ne))
            e0 += ne
    pieces.sort(key=lambda p: -el(p[0], p[2] - p[1]))
    cores = [[] for _ in range(NCORES)]
    loads = [0.0] * NCORES
    for p in pieces:
        c = min(range(NCORES), key=lambda i: loads[i])
        cores[c].append(p)
        loads[c] += el(p[0], p[2] - p[1])
    for _ in range(64):  # shave the max core onto the min core
        cM = max(range(NCORES), key=lambda i: loads[i])
        cm = min(range(NCORES), key=lambda i: loads[i])
        surplus = loads[cM] - loads[cm]
        best = None
        for idx, (g, e0, e1) in enumerate(cores[cM]):
            ne_mv = int((surplus / 2 - GFIX) / (Ba[g] + OUT_W))
            ne_mv = min(ne_mv, e1 - e0 - 64)
            if ne_mv >= 64 and (best is None or ne_mv > best[1]):
                best = (idx, ne_mv)
        if best is None:
            break
        idx, ne_mv = best
        g, e0, e1 = cores[cM][idx]
        cores[cM][idx] = (g, e0, e1 - ne_mv)
        cores[cm].append((g, e1 - ne_mv, e1))
        loads[cM] -= (Ba[g] + OUT_W) * ne_mv
        loads[cm] += el(g, ne_mv)
    return La, Ba, cores


def kernel(x, SCB, edge_index, edge_len, beta_len,
           W1, b1, W2, b2, W3, b3, W4, b4, W5, b5, W6, b6):
    x = np.asarray(x, np.float32)
    SCB = np.asarray(SCB, np.float32)
    edge_index = np.asarray(edge_index, np.int32)
    La, Ba, cores = _plan(np.asarray(edge_len), np.asarray(beta_len))
    ngmax = max(len(c) for c in cores)

    # ---- host-side packing (data movement only) ----
    ef_all = []
    for b in range(B):
        src = edge_index[b, 0, : La[b]]
        dst = edge_index[b, 1, : La[b]]
        ef_all.append(np.concatenate([x[b][src], x[b][dst]], axis=1))  # [La,4]

    ef_off = [[0] * ngmax for _ in range(NCORES)]
    emax = 1
    for c in range(NCORES):
        eo = 0
        for i, (g, e0, e1) in enumerate(cores[c]):
            ef_off[c][i] = eo
            eo += La[g]
        emax = max(emax, eo)

    W3b = np.ascontiguousarray(W3[64:], np.float32)       # [4,128]
    exp64 = np.zeros((64, 128), np.float32)               # expand 32b -> 4x32p
    for i in range(64):
        exp64[i, 4 * (i % 32) : 4 * (i % 32) + 4] = 1.0
    w3bm = np.zeros((128, 32 * 128), np.float32)          # masked stacked W3b
    for i in range(32):
        w3bm[4 * i : 4 * i + 4, i * 128 : (i + 1) * 128] = W3b

    # packed constants: one fp32 tensor + one bf16 tensor -> 2 DMAs
    # fp32 [128, 518]: w1(0:64) w2(64:128) w3a(128:256) w4(256:384)
    #   ident(384:512) b1c..b6c,b4x64(512:518)
    CF = 518
    constf = np.zeros((128, CF), np.float32)
    constf[:4, 0:64] = W1
    constf[:64, 64:128] = W2
    constf[:64, 128:256] = W3[:64]
    constf[:, 256:384] = W4
    constf[:, 384:512] = np.eye(128, dtype=np.float32)
    constf[:64, 512] = np.asarray(b1, np.float32)
    constf[:64, 513] = np.asarray(b2, np.float32)
    constf[:, 514] = np.asarray(b3, np.float32)
    constf[:, 515] = 64.0 * np.asarray(b4, np.float32)
    constf[:, 516] = np.asarray(b5, np.float32)
    constf[:, 517] = np.asarray(b6, np.float32)
    # bf16 [128, 4736]: w3bm(0:4096) w4b(4096:4224) w5b(4224:4352)
    #   w6b(4352:4480) exp64b(4480:4608) identb(4608:4736)
    CB = 4736
    constb = np.zeros((128, CB), np.float32)
    constb[:, 0:4096] = w3bm
    constb[:, 4096:4224] = W4
    constb[:, 4224:4352] = W5
    constb[:, 4352:4480] = W6
    constb[:64, 4480:4608] = exp64
    constb[:, 4608:4736] = np.eye(128, dtype=np.float32)
    constb = constb.astype(NPBF16)

    in_maps = []
    for c in range(NCORES):
        scb_pack = np.zeros((ngmax * 64, MAX_E), np.float32)
        eft32 = np.zeros((128, emax), np.float32)
        for i, (g, e0, e1) in enumerate(cores[c]):
            scb_pack[i * 64 : i * 64 + 64, : La[g]] = SCB[g][:, : La[g]]
            eft32[:, ef_off[c][i] : ef_off[c][i] + La[g]] = \
                np.tile(ef_all[g].T, (32, 1))
        in_maps.append({
            "constf": constf,
            "constb": constb,
            "scb_pack": scb_pack.astype(NPBF16),
            "eft32": eft32.astype(NPBF16),
        })

    # ---- build program ----
    nc = bacc.Bacc("TRN2", target_bir_lowering=False, debug=False,
                   num_devices=NCORES)
    d_in = {}
    for name, arr in in_maps[0].items():
        dt = BF16 if arr.dtype == NPBF16 else F32
        d_in[name] = nc.dram_tensor(name, list(arr.shape), dt,
                                    kind="ExternalInput")
    d_out = nc.dram_tensor("out", [ngmax * MAX_E, HID], F32,
                           kind="ExternalOutput")

    with tile.TileContext(nc) as tc:
        pid = nc.partition_id()
        with (
            tc.tile_pool(name="const", bufs=1) as cpool,
            tc.tile_pool(name="sbA", bufs=2) as sbA,
            tc.tile_pool(name="sbB", bufs=3) as sbB,
            tc.tile_pool(name="psS", bufs=2, space="PSUM") as psS,
            tc.tile_pool(name="psG", bufs=2, space="PSUM") as psG,
            tc.tile_pool(name="psH", bufs=2, space="PSUM") as psH,
            tc.tile_pool(name="psO", bufs=1, space="PSUM") as psO,
        ):
            cf = cpool.tile([128, CF], F32, tag="constf")
            nc.sync.dma_start(cf[:], d_in["constf"].ap())
            cb = cpool.tile([128, CB], BF16, tag="constb")
            nc.sync.dma_start(cb[:], d_in["constb"].ap())
            eftc = cpool.tile([128, emax], BF16, tag="eft32")
            nc.sync.dma_start(eftc[:], d_in["eft32"].ap())
            cst = {
                "w1": cf[:4, 0:64], "w2": cf[:64, 64:128],
                "w3a": cf[:64, 128:256], "w4": cf[:, 256:384],
                "ident": cf[:, 384:512],
                "b1c": cf[:64, 512:513], "b2c": cf[:64, 513:514],
                "b3c": cf[:, 514:515], "b4x64": cf[:, 515:516],
                "b5c": cf[:, 516:517], "b6c": cf[:, 517:518],
                "w3bm": cb[:, 0:4096], "w4b": cb[:, 4096:4224],
                "w5b": cb[:, 4224:4352], "w6b": cb[:, 4352:4480],
                "exp64b": cb[:64, 4480:4608], "identb": cb[:, 4608:4736],
            }

            def build_graph(c, slot, g, es, ee):
                la, ba = La[g], Ba[g]
                nech = (la + 127) // 128  # 128-e chunks for transposes / s
                goff_e = ef_off[c][slot]

                # ---- stage A ----
                scb_sb = sbA.tile([64, MAX_E], BF16, tag="scb")
                nc.sync.dma_start(
                    scb_sb[:, :la],
                    d_in["scb_pack"].ap()[slot * 64 : slot * 64 + 64, :la],
                )
                # |SCB| in-place (ACT Abs); rows >= ba never read
                nc.scalar.activation(scb_sb[:ba, :la], scb_sb[:ba, :la],
                                     AF.Abs, bias=0.0, scale=1.0)
                # scb_T chunks [128e, 64b] via PE transpose (for s)
                scbT_sb = sbA.tile([128, 64 * 8], BF16, tag="scbT")
                for ec in range(nech):
                    n = min(128, la - ec * 128)
                    tp = psO.tile([128, 64], BF16, tag="pM")
                    nc.tensor.transpose(
                        tp[:n, :], scb_sb[:, ec * 128 : ec * 128 + n],
                        cst["identb"][:64, :64],
                    )
                    nc.vector.tensor_copy(scbT_sb[:n, ec * 64 : ec * 64 + 64],
                                          tp[:n, :])
                # ef rows [e,4] per 128-chunk (for s)
                efr_sb = sbA.tile([128, 4 * 8], BF16, tag="efr")
                for ec in range(nech):
                    n = min(128, la - ec * 128)
                    tp2 = psO.tile([128, 4], BF16, tag="pM")
                    nc.tensor.transpose(
                        tp2[:n, :],
                        eftc[0:4, goff_e + ec * 128 : goff_e + ec * 128 + n],
                        cst["identb"][:4, :4],
                    )
                    nc.vector.tensor_copy(efr_sb[:n, ec * 4 : ec * 4 + 4],
                                          tp2[:n, :])
                # s_T[k, beta] = sum_e ef[e,k] |scb|_T[e, beta]
                ps_s = psO.tile([4, 128], F32, tag="pM")
                for ec in range(nech):
                    n = min(128, la - ec * 128)
                    nc.tensor.matmul(
                        ps_s[:, :ba],
                        efr_sb[:n, ec * 4 : ec * 4 + 4],
                        scbT_sb[:n, ec * 64 : ec * 64 + ba],
                        start=(ec == 0), stop=(ec == nech - 1),
                    )
                s_sb = sbA.tile([4, 65], F32, tag="s")
                nc.vector.memset(s_sb[:], 0.0)
                nc.vector.tensor_copy(s_sb[:, :ba], ps_s[:, :ba])
                # emb / A chain (one padded col at index ba -> A_pad)
                nb = ba + 1
                pe1 = psO.tile([64, 65], F32, tag="pM")
                nc.tensor.matmul(pe1[:, :nb], cst["w1"], s_sb[:, :nb],
                                 start=True, stop=True)
                e1_sb = sbA.tile([64, 65], F32, tag="e1")
                nc.scalar.activation(e1_sb[:, :nb], pe1[:, :nb], AF.Relu,
                                     bias=cst["b1c"], scale=1.0)
                pe2 = psO.tile([64, 65], F32, tag="pM")
                nc.tensor.matmul(pe2[:, :nb], cst["w2"], e1_sb[:, :nb],
                                 start=True, stop=True)
                e2_sb = sbA.tile([64, 65], F32, tag="e2")
                nc.scalar.activation(e2_sb[:, :nb], pe2[:, :nb], AF.Identity,
                                     bias=cst["b2c"], scale=1.0)
                pa = psO.tile([128, 65], F32, tag="pM")
                nc.tensor.matmul(pa[:, :nb], cst["w3a"], e2_sb[:, :nb],
                                 start=True, stop=True)
                A_sb = sbA.tile([128, 65], F32, tag="A")
                nc.scalar.activation(A_sb[:, :nb], pa[:, :nb], AF.Identity,
                                     bias=cst["b3c"], scale=1.0)
                # K0 = relu(A_pad); vb = (64-Ba) * K0@W4 + 64*b4
                K0_sb = sbA.tile([128, 1], F32, tag="K0")
                nc.scalar.activation(K0_sb[:], A_sb[:, ba : ba + 1], AF.Relu,
                                     bias=0.0, scale=1.0)
                pk = psO.tile([128, 1], F32, tag="pM")
                nc.tensor.matmul(pk[:], cst["w4"], K0_sb[:],
                                 start=True, stop=True)
                vb_sb = sbA.tile([128, 1], F32, tag="vb")
                nc.scalar.activation(vb_sb[:], pk[:], AF.Identity,
                                     bias=cst["b4x64"],
                                     scale=float(64 - ba))

                # ---- stage B + out stage, per 512-e chunk ----
                pe_acc = [b for b in range(ba) if not _acc_on_dve(b)]
                dve_acc = [b for b in range(ba) if _acc_on_dve(b)]
                if not pe_acc:  # guarantee pW4 gets a start
                    pe_acc = [dve_acc.pop(0)]
                for e0 in range(es, ee, ECHUNK):
                    n = min(ECHUNK, ee - e0)
                    ecol = goff_e + e0  # column of this chunk in eft32
                    pW4 = psH.tile([128, ECHUNK], F32, tag="H")
                    H2 = sbB.tile([128, ECHUNK], F32, tag="H2")
                    h2_open = False
                    for g0 in range(0, ba, 32):
                        gsz = min(32, ba - g0)
                        P1 = psS.tile([128, ECHUNK], F32, tag="P1")
                        nc.tensor.matmul(
                            P1[:, :n],
                            cst["exp64b"][g0 : g0 + gsz, :],
                            scb_sb[g0 : g0 + gsz, e0 : e0 + n],
                            start=True, stop=True)
                        efs = sbB.tile([128, ECHUNK], BF16, tag="efs")
                        nc.vector.tensor_mul(efs[:, :n], P1[:, :n],
                                             eftc[:, ecol : ecol + n])
                        for i in range(gsz):
                            b = g0 + i
                            pG = psG.tile([128, ECHUNK], F32, tag="pG")
                            nc.tensor.matmul(
                                pG[:, :n],
                                cst["w3bm"][:, i * 128 : (i + 1) * 128],
                                efs[:, :n], start=True, stop=True)
                            r = sbB.tile([128, ECHUNK], BF16, tag="r")
                            if _relu_on_dve(b):
                                nc.vector.tensor_scalar(
                                    r[:, :n], pG[:, :n],
                                    A_sb[:, b : b + 1], 0.0,
                                    ALU.add, ALU.max)
                            else:
                                nc.scalar.activation(
                                    r[:, :n], pG[:, :n], AF.Relu,
                                    bias=A_sb[:, b : b + 1], scale=1.0)
                            if b in pe_acc:
                                nc.tensor.matmul(
                                    pW4[:, :n], cst["w4b"], r[:, :n],
                                    start=(b == pe_acc[0]),
                                    stop=(b == pe_acc[-1] and not dve_acc))
                            elif not h2_open:
                                nc.vector.tensor_copy(H2[:, :n], r[:, :n])
                                h2_open = True
                            else:
                                nc.vector.tensor_add(H2[:, :n], H2[:, :n],
                                                     r[:, :n])
                    if dve_acc:
                        H2b = sbB.tile([128, ECHUNK], BF16, tag="H2b")
                        nc.vector.tensor_copy(H2b[:, :n], H2[:, :n])
                        nc.tensor.matmul(pW4[:, :n], cst["w4b"],
                                         H2b[:, :n], start=False, stop=True)
                    # out stage: h = pW4 + vb; relu(h@W5+b5)@W6+b6
                    r5 = sbB.tile([128, ECHUNK], BF16, tag="r5")
                    nc.scalar.activation(r5[:, :n], pW4[:, :n], AF.Identity,
                                         bias=vb_sb[:], scale=1.0)
                    p2 = psO.tile([128, ECHUNK], F32, tag="pO")
                    nc.tensor.matmul(p2[:, :n], cst["w5b"], r5[:, :n],
                                     start=True, stop=True)
                    r6 = sbB.tile([128, ECHUNK], BF16, tag="r6")
                    nc.scalar.activation(r6[:, :n], p2[:, :n], AF.Relu,
                                         bias=cst["b5c"], scale=1.0)
                    p3 = psO.tile([128, ECHUNK], F32, tag="pO")
                    nc.tensor.matmul(p3[:, :n], cst["w6b"], r6[:, :n],
                                     start=True, stop=True)
                    o_sb = sbB.tile([128, ECHUNK], F32, tag="o")
                    nc.scalar.activation(o_sb[:, :n], p3[:, :n], AF.Identity,
                                         bias=cst["b6c"], scale=1.0)
                    oT = sbB.tile([128, ECHUNK], F32, tag="oT")
                    for et in range(0, n, 128):
                        m = min(128, n - et)
                        po = psO.tile([128, 128], F32, tag="pM")
                        nc.tensor.transpose(po[:m, :], o_sb[:, et : et + m],
                                            cst["ident"])
                        nc.scalar.copy(oT[:m, et : et + 128], po[:m, :])
                    r0 = slot * MAX_E + e0
                    nfull = (n // 128) * 128
                    if nfull:
                        dst = d_out.ap()[r0 : r0 + nfull, :].rearrange(
                            "(et p) h -> p et h", p=128)
                        nc.sync.dma_start(dst, oT[:, :nfull])
                    if n > nfull:
                        m = n - nfull
                        nc.sync.dma_start(
                            d_out.ap()[r0 + nfull : r0 + n, :],
                            oT[:m, nfull : nfull + 128])

            def build_core(c):
                for slot, (g, e0, e1) in enumerate(cores[c]):
                    build_graph(c, slot, g, e0, e1)

            def dispatch(lo, hi):
                if hi - lo == 1:
                    build_core(lo)
                    return
                mid = (lo + hi) // 2
                with tc.If(pid < mid) as cmp:
                    dispatch(lo, mid)
                with cmp.Else():
                    dispatch(mid, hi)

            dispatch(0, NCORES)

    import os
    if os.environ.get("KERNEL_BUILD_ONLY"):
        return np.zeros((B * MAX_E, HID), np.float32)
    nc.compile()
    if os.environ.get("KERNEL_COMPILE_ONLY"):
        import tempfile
        neff = bass_utils.compile_bass_kernel(nc, tempfile.mkdtemp())
        print("NEFF:", neff)
        return np.zeros((B * MAX_E, HID), np.float32)
    trace = bool(os.environ.get("KERNEL_TRACE"))
    res = bass_utils.run_bass_kernel_spmd(
        nc, in_maps, core_ids=list(range(NCORES)),
        trace=trace,
        trace_cores=list(range(NCORES)) if trace else None,
    )
    global LAST_EXEC_NS, LAST_RESULTS
    LAST_RESULTS = res
    LAST_EXEC_NS = res.exec_time_ns

    out = np.zeros((B * MAX_E, HID), np.float32)
    for c in range(NCORES):
        oc = res.results[c]["out"]
        for slot, (g, e0, e1) in enumerate(cores[c]):
            out[g * MAX_E + e0 : g * MAX_E + e1] = \
                oc[slot * MAX_E + e0 : slot * MAX_E + e1]
    return out


# revision 16
# speedup vs baseline: 2.8661x; 1.0023x over previous
"""Trainium2 Bass kernel for nn_CycleNet_EPD (ragged graph edge-phase decoder).

Math (per graph b, with La = edge_len[b], Ba = beta_len[b]):
  ef[e,:4]   = [x[src_e], x[dst_e]]                        (edge features)
  s[beta,:]  = sum_e |SCB[b,beta,e]| * ef[e,:]             (beta < Ba, e < La)
  emb        = relu(s@W1+b1)@W2+b2                         [Ba,64]
  A[beta,:]  = emb@W3a + b3                                [Ba,128]  (W3a=W3[:64])
  G[e,:]     = ef@W3b                                      [La,128]  (W3b=W3[64:])
  H[e,:]     = sum_{beta<Ba} relu(A[beta,:] + |SCB[b,beta,e]|*G[e,:])
  out[e,:]   = relu((H@W4 + vb)@W5+b5)@W6+b6
               vb = 64*b4 + (64-Ba)*relu(A_pad)@W4  (A_pad: padded-beta row)
  rows with e >= La are zero.

Device mapping (per graph), [h, e] layout, e-chunks of 512:
  - G is rank 4 (G = W3b^T ef), so scb_beta (x) G = W3b^T (ef (x) scb_beta).
    Per 32-beta group one PE "expand" matmul (bf16) broadcasts scb rows to
    4x32 partitions; one DVE multiply with the host-shipped 32x-tiled edge
    features (eft32) builds EFS[4i+k,e] = ef[k,e]*|scb|[g0+i,e] in bf16.
    Per beta, a single K=128 matmul with masked stacked weights w3bm
    (rows 4i..4i+3 = W3b, zeros elsewhere) yields scb_beta*G in PSUM.
  - relu(+A bias) on ACT (majority) / DVE tensor_scalar (minority, balance).
  - The beta-sum is folded into W4: r@W4b accumulates into one PSUM bank
    (start/stop over the chunk's betas); a minority of betas accumulate r on
    DVE into H2 which is flushed through W4b with one extra matmul.
  - out stage: W5/W6 bf16 matmuls with ACT relu/bias epilogues, PE transpose
    to [e, 128], one batched DMA per chunk to DRAM.
  - all hot-loop matmuls bf16 (1 cyc/row); fp32 only in the tiny emb chain.

Sharding: per-core work items (graph, e0, e1); effective load model counts
columns Ba*ne plus per-slice and per-edge-column fixed overheads so cores
with many small graphs are not overloaded. One NEFF; each core's ragged
schedule sits in its own branch of a partition-id If-tree.
Host does only data movement: gather of x rows by edge_index, packing /
replication / dtype casts of inputs, and scatter of per-core outputs into
the full [B*MAX_E, HID] result (padded rows stay zero).
"""

import sys

sys.path.insert(0, "/opt/trn_rl_repo")

import ml_dtypes
import numpy as np

import concourse.bacc as bacc
import concourse.mybir as mybir
import concourse.tile as tile
from concourse import bass_utils

B, MAX_N, MAX_E, MAX_BETA = 16, 512, 1024, 64
NODE_F, HID = 2, 128
NCORES = 8
F32 = mybir.dt.float32
BF16 = mybir.dt.bfloat16
AF = mybir.ActivationFunctionType
ALU = mybir.AluOpType
NPBF16 = ml_dtypes.bfloat16

ECHUNK = 512   # e-tile for stage B / out stage (one PSUM bank)
GFIX = 3000    # planner: per-slice fixed cost (stage A), in column units
OUT_W = 5      # planner: out-stage cost per edge column, in column units


def _relu_on_dve(b):
    return b % 5 == 2


def _acc_on_dve(b):
    return b % 5 in (1, 3)


def _plan(edge_len, beta_len):
    """Per-core work items (g, e0, e1); large graphs split by edge range.

    Effective load = (Ba + OUT_W) * ne + GFIX per slice, balancing stage-B
    columns plus out-stage and per-graph fixed overheads.  Split oversized
    graphs, LPT-assign pieces, then iteratively shave edges from the max
    core onto the min core."""
    La = [max(1, min(MAX_E, int(v))) for v in edge_len]
    Ba = [max(1, min(MAX_BETA, int(v))) for v in beta_len]

    def el(g, ne):
        return (Ba[g] + OUT_W) * ne + GFIX

    total = sum(el(g, La[g]) for g in range(B))
    target = total / NCORES
    pieces = []
    for g in range(B):
        k = max(1, min(round(el(g, La[g]) / target + 0.25), -(-La[g] // 64)))
        base, rem = divmod(La[g], k)
        e0 = 0
        for j in range(k):
            ne = base + (1 if j < rem else 0)
            pieces.append((g, e0, e0 + ne))
            e0 += ne
    pieces.sort(key=lambda p: -el(p[0], p[2] - p[1]))
    cores = [[] for _ in range(NCORES)]
    loads = [0.0] * NCORES
    for p in pieces:
        c = min(range(NCORES), key=lambda i: loads[i])
        cores[c].append(p)
        loads[c] += el(p[0], p[2] - p[1])
    for _ in range(64):  # shave the max core onto the min core
        cM = max(range(NCORES), key=lambda i: loads[i])
        cm = min(range(NCORES), key=lambda i: loads[i])
        surplus = loads[cM] - loads[cm]
        best = None
        for idx, (g, e0, e1) in enumerate(cores[cM]):
            ne_mv = int((surplus / 2 - GFIX) / (Ba[g] + OUT_W))
            ne_mv = min(ne_mv, e1 - e0 - 64)
            if ne_mv >= 64 and (best is None or ne_mv > best[1]):
                best = (idx, ne_mv)
        if best is None:
            break
        idx, ne_mv = best
        g, e0, e1 = cores[cM][idx]
        cores[cM][idx] = (g, e0, e1 - ne_mv)
        cores[cm].append((g, e1 - ne_mv, e1))
        loads[cM] -= (Ba[g] + OUT_W) * ne_mv
        loads[cm] += el(g, ne_mv)
    return La, Ba, cores


def kernel(x, SCB, edge_index, edge_len, beta_len,
           W1, b1, W2, b2, W3, b3, W4, b4, W5, b5, W6, b6):
    x = np.asarray(x, np.float32)
    SCB = np.asarray(SCB, np.float32)
    edge_index = np.asarray(edge_index, np.int32)
    La, Ba, cores = _plan(np.asarray(edge_len), np.asarray(beta_len))
    ngmax = max(len(c) for c in cores)

    # ---- host-side packing (data movement only) ----
    ef_all = []
    for b in range(B):
        src = edge_index[b, 0, : La[b]]
        dst = edge_index[b, 1, : La[b]]
        ef_all.append(np.concatenate([x[b][src], x[b][dst]], axis=1))  # [La,4]

    ef_off = [[0] * ngmax for _ in range(NCORES)]
    emax = 1
    for c in range(NCORES):
        eo = 0
        for i, (g, e0, e1) in enumerate(cores[c]):
            ef_off[c][i] = eo
            eo += La[g]
        emax = max(emax, eo)

    W3b = np.ascontiguousarray(W3[64:], np.float32)       # [4,128]
    exp64 = np.zeros((64, 128), np.float32)               # expand 32b -> 4x32p
    for i in range(64):
        exp64[i, 4 * (i % 32) : 4 * (i % 32) + 4] = 1.0
    w3bm = np.zeros((128, 32 * 128), np.float32)          # masked stacked W3b
    for i in range(32):
        w3bm[4 * i : 4 * i + 4, i * 128 : (i + 1) * 128] = W3b

    # packed constants: one fp32 tensor + one bf16 tensor -> 2 DMAs
    # fp32 [128, 518]: w1(0:64) w2(64:128) w3a(128:256) w4(256:384)
    #   ident(384:512) b1c..b6c,b4x64(512:518)
    CF = 518
    constf = np.zeros((128, CF), np.float32)
    constf[:4, 0:64] = W1
    constf[:64, 64:128] = W2
    constf[:64, 128:256] = W3[:64]
    constf[:, 256:384] = W4
    constf[:, 384:512] = np.eye(128, dtype=np.float32)
    constf[:64, 512] = np.asarray(b1, np.float32)
    constf[:64, 513] = np.asarray(b2, np.float32)
    constf[:, 514] = np.asarray(b3, np.float32)
    constf[:, 515] = 64.0 * np.asarray(b4, np.float32)
    constf[:, 516] = np.asarray(b5, np.float32)
    constf[:, 517] = np.asarray(b6, np.float32)
    # bf16 [128, 4736]: w3bm(0:4096) w4b(4096:4224) w5b(4224:4352)
    #   w6b(4352:4480) exp64b(4480:4608) identb(4608:4736)
    CB = 4736
    constb = np.zeros((128, CB), np.float32)
    constb[:, 0:4096] = w3bm
    constb[:, 4096:4224] = W4
    constb[:, 4224:4352] = W5
    constb[:, 4352:4480] = W6
    constb[:64, 4480:4608] = exp64
    constb[:, 4608:4736] = np.eye(128, dtype=np.float32)
    constb = constb.astype(NPBF16)

    in_maps = []
    for c in range(NCORES):
        scb_pack = np.zeros((64, ngmax * MAX_E), np.float32)
        eft32 = np.zeros((128, emax), np.float32)
        for i, (g, e0, e1) in enumerate(cores[c]):
            scb_pack[:, i * MAX_E : i * MAX_E + La[g]] = SCB[g][:, : La[g]]
            eft32[:, ef_off[c][i] : ef_off[c][i] + La[g]] = \
                np.tile(ef_all[g].T, (32, 1))
        in_maps.append({
            "constf": constf,
            "constb": constb,
            "scb_pack": scb_pack.astype(NPBF16),
            "eft32": eft32.astype(NPBF16),
        })

    # ---- build program ----
    nc = bacc.Bacc("TRN2", target_bir_lowering=False, debug=False,
                   num_devices=NCORES)
    d_in = {}
    for name, arr in in_maps[0].items():
        dt = BF16 if arr.dtype == NPBF16 else F32
        d_in[name] = nc.dram_tensor(name, list(arr.shape), dt,
                                    kind="ExternalInput")
    d_out = nc.dram_tensor("out", [ngmax * MAX_E, HID], F32,
                           kind="ExternalOutput")

    with tile.TileContext(nc) as tc:
        pid = nc.partition_id()
        with (
            tc.tile_pool(name="const", bufs=1) as cpool,
            tc.tile_pool(name="sbA", bufs=2) as sbA,
            tc.tile_pool(name="sbB", bufs=3) as sbB,
            tc.tile_pool(name="psS", bufs=2, space="PSUM") as psS,
            tc.tile_pool(name="psG", bufs=2, space="PSUM") as psG,
            tc.tile_pool(name="psH", bufs=2, space="PSUM") as psH,
            tc.tile_pool(name="psO", bufs=1, space="PSUM") as psO,
        ):
            cf = cpool.tile([128, CF], F32, tag="constf")
            nc.sync.dma_start(cf[:], d_in["constf"].ap())
            cb = cpool.tile([128, CB], BF16, tag="constb")
            nc.sync.dma_start(cb[:], d_in["constb"].ap())
            eftc = cpool.tile([128, emax], BF16, tag="eft32")
            nc.sync.dma_start(eftc[:], d_in["eft32"].ap())
            cst = {
                "w1": cf[:4, 0:64], "w2": cf[:64, 64:128],
                "w3a": cf[:64, 128:256], "w4": cf[:, 256:384],
                "ident": cf[:, 384:512],
                "b1c": cf[:64, 512:513], "b2c": cf[:64, 513:514],
                "b3c": cf[:, 514:515], "b4x64": cf[:, 515:516],
                "b5c": cf[:, 516:517], "b6c": cf[:, 517:518],
                "w3bm": cb[:, 0:4096], "w4b": cb[:, 4096:4224],
                "w5b": cb[:, 4224:4352], "w6b": cb[:, 4352:4480],
                "exp64b": cb[:64, 4480:4608], "identb": cb[:, 4608:4736],
            }

            def build_graph(c, slot, g, es, ee):
                la, ba = La[g], Ba[g]
                nech = (la + 127) // 128  # 128-e chunks for transposes / s
                goff_e = ef_off[c][slot]

                # ---- stage A ----
                scb_sb = sbA.tile([64, MAX_E], BF16, tag="scb")
                nc.sync.dma_start(
                    scb_sb[:, :la],
                    d_in["scb_pack"].ap()[:, slot * MAX_E : slot * MAX_E + la],
                )
                # |SCB| in-place (ACT Abs); rows >= ba never read
                nc.scalar.activation(scb_sb[:ba, :la], scb_sb[:ba, :la],
                                     AF.Abs, bias=0.0, scale=1.0)
                # scb_T chunks [128e, 64b] via PE transpose (for s)
                scbT_sb = sbA.tile([128, 64 * 8], BF16, tag="scbT")
                for ec in range(nech):
                    n = min(128, la - ec * 128)
                    tp = psO.tile([128, 64], BF16, tag="pM")
                    nc.tensor.transpose(
                        tp[:n, :], scb_sb[:, ec * 128 : ec * 128 + n],
                        cst["identb"][:64, :64],
                    )
                    nc.vector.tensor_copy(scbT_sb[:n, ec * 64 : ec * 64 + 64],
                                          tp[:n, :])
                # ef rows [e,4] per 128-chunk (for s)
                efr_sb = sbA.tile([128, 4 * 8], BF16, tag="efr")
                for ec in range(nech):
                    n = min(128, la - ec * 128)
                    tp2 = psO.tile([128, 4], BF16, tag="pM")
                    nc.tensor.transpose(
                        tp2[:n, :],
                        eftc[0:4, goff_e + ec * 128 : goff_e + ec * 128 + n],
                        cst["identb"][:4, :4],
                    )
                    nc.vector.tensor_copy(efr_sb[:n, ec * 4 : ec * 4 + 4],
                                          tp2[:n, :])
                # s_T[k, beta] = sum_e ef[e,k] |scb|_T[e, beta]
                ps_s = psO.tile([4, 128], F32, tag="pM")
                for ec in range(nech):
                    n = min(128, la - ec * 128)
                    nc.tensor.matmul(
                        ps_s[:, :ba],
                        efr_sb[:n, ec * 4 : ec * 4 + 4],
                        scbT_sb[:n, ec * 64 : ec * 64 + ba],
                        start=(ec == 0), stop=(ec == nech - 1),
                    )
                s_sb = sbA.tile([4, 65], F32, tag="s")
                nc.vector.memset(s_sb[:], 0.0)
                nc.vector.tensor_copy(s_sb[:, :ba], ps_s[:, :ba])
                # emb / A chain (one padded col at index ba -> A_pad)
                nb = ba + 1
                pe1 = psO.tile([64, 65], F32, tag="pM")
                nc.tensor.matmul(pe1[:, :nb], cst["w1"], s_sb[:, :nb],
                                 start=True, stop=True)
                e1_sb = sbA.tile([64, 65], F32, tag="e1")
                nc.scalar.activation(e1_sb[:, :nb], pe1[:, :nb], AF.Relu,
                                     bias=cst["b1c"], scale=1.0)
                pe2 = psO.tile([64, 65], F32, tag="pM")
                nc.tensor.matmul(pe2[:, :nb], cst["w2"], e1_sb[:, :nb],
                                 start=True, stop=True)
                e2_sb = sbA.tile([64, 65], F32, tag="e2")
                nc.scalar.activation(e2_sb[:, :nb], pe2[:, :nb], AF.Identity,
                                     bias=cst["b2c"], scale=1.0)
                pa = psO.tile([128, 65], F32, tag="pM")
                nc.tensor.matmul(pa[:, :nb], cst["w3a"], e2_sb[:, :nb],
                                 start=True, stop=True)
                A_sb = sbA.tile([128, 65], F32, tag="A")
                nc.scalar.activation(A_sb[:, :nb], pa[:, :nb], AF.Identity,
                                     bias=cst["b3c"], scale=1.0)
                # K0 = relu(A_pad); vb = (64-Ba) * K0@W4 + 64*b4
                K0_sb = sbA.tile([128, 1], F32, tag="K0")
                nc.scalar.activation(K0_sb[:], A_sb[:, ba : ba + 1], AF.Relu,
                                     bias=0.0, scale=1.0)
                pk = psO.tile([128, 1], F32, tag="pM")
                nc.tensor.matmul(pk[:], cst["w4"], K0_sb[:],
                                 start=True, stop=True)
                vb_sb = sbA.tile([128, 1], F32, tag="vb")
                nc.scalar.activation(vb_sb[:], pk[:], AF.Identity,
                                     bias=cst["b4x64"],
                                     scale=float(64 - ba))

                # ---- stage B + out stage, per 512-e chunk ----
                pe_acc = [b for b in range(ba) if not _acc_on_dve(b)]
                dve_acc = [b for b in range(ba) if _acc_on_dve(b)]
                if not pe_acc:  # guarantee pW4 gets a start
                    pe_acc = [dve_acc.pop(0)]
                for e0 in range(es, ee, ECHUNK):
                    n = min(ECHUNK, ee - e0)
                    ecol = goff_e + e0  # column of this chunk in eft32
                    pW4 = psH.tile([128, ECHUNK], F32, tag="H")
                    H2 = sbB.tile([128, ECHUNK], F32, tag="H2")
                    h2_open = False
                    for g0 in range(0, ba, 32):
                        gsz = min(32, ba - g0)
                        P1 = psS.tile([128, ECHUNK], F32, tag="P1")
                        nc.tensor.matmul(
                            P1[:, :n],
                            cst["exp64b"][g0 : g0 + gsz, :],
                            scb_sb[g0 : g0 + gsz, e0 : e0 + n],
                            start=True, stop=True)
                        efs = sbB.tile([128, ECHUNK], BF16, tag="efs")
                        nc.vector.tensor_mul(efs[:, :n], P1[:, :n],
                                             eftc[:, ecol : ecol + n])
                        for i in range(gsz):
                            b = g0 + i
                            pG = psG.tile([128, ECHUNK], F32, tag="pG")
                            nc.tensor.matmul(
                                pG[:, :n],
                                cst["w3bm"][:, i * 128 : (i + 1) * 128],
                                efs[:, :n], start=True, stop=True)
                            r = sbB.tile([128, ECHUNK], BF16, tag="r")
                            if _relu_on_dve(b):
                                nc.vector.tensor_scalar(
                                    r[:, :n], pG[:, :n],
                                    A_sb[:, b : b + 1], 0.0,
                                    ALU.add, ALU.max)
                            else:
                                nc.scalar.activation(
                                    r[:, :n], pG[:, :n], AF.Relu,
                                    bias=A_sb[:, b : b + 1], scale=1.0)
                            if b in pe_acc:
                                nc.tensor.matmul(
                                    pW4[:, :n], cst["w4b"], r[:, :n],
                                    start=(b == pe_acc[0]),
                                    stop=(b == pe_acc[-1] and not dve_acc))
                            elif not h2_open:
                                nc.vector.tensor_copy(H2[:, :n], r[:, :n])
                                h2_open = True
                            else:
                                nc.vector.tensor_add(H2[:, :n], H2[:, :n],
                                                     r[:, :n])
                    if dve_acc:
                        H2b = sbB.tile([128, ECHUNK], BF16, tag="H2b")
                        nc.vector.tensor_copy(H2b[:, :n], H2[:, :n])
                        nc.tensor.matmul(pW4[:, :n], cst["w4b"],
                                         H2b[:, :n], start=False, stop=True)
                    # out stage: h = pW4 + vb; relu(h@W5+b5)@W6+b6
                    r5 = sbB.tile([128, ECHUNK], BF16, tag="r5")
                    nc.scalar.activation(r5[:, :n], pW4[:, :n], AF.Identity,
                                         bias=vb_sb[:], scale=1.0)
                    p2 = psO.tile([128, ECHUNK], F32, tag="pO")
                    nc.tensor.matmul(p2[:, :n], cst["w5b"], r5[:, :n],
                                     start=True, stop=True)
                    r6 = sbB.tile([128, ECHUNK], BF16, tag="r6")
                    nc.scalar.activation(r6[:, :n], p2[:, :n], AF.Relu,
                                         bias=cst["b5c"], scale=1.0)
                    p3 = psO.tile([128, ECHUNK], F32, tag="pO")
                    nc.tensor.matmul(p3[:, :n], cst["w6b"], r6[:, :n],
                                     start=True, stop=True)
                    o_sb = sbB.tile([128, ECHUNK], F32, tag="o")
                    nc.scalar.activation(o_sb[:, :n], p3[:, :n], AF.Identity,
                                         bias=cst["b6c"], scale=1.0)
                    oT = sbB.tile([128, ECHUNK], F32, tag="oT")
                    for et in range(0, n, 128):
                        m = min(128, n - et)
                        po = psO.tile([128, 128], F32, tag="pM")
                        nc.tensor.transpose(po[:m, :], o_sb[:, et : et + m],
                                            cst["ident"])
                        nc.scalar.copy(oT[:m, et : et + 128], po[:m, :])
                    r0 = slot * MAX_E + e0
                    nfull = (n // 128) * 128
                    if nfull:
                        dst = d_out.ap()[r0 : r0 + nfull, :].rearrange(
                            "(et p) h -> p et h", p=128)
                        nc.sync.dma_start(dst, oT[:, :nfull])
                    if n > nfull:
                        m = n - nfull
                        nc.sync.dma_start(
                            d_out.ap()[r0 + nfull : r0 + n, :],
                            oT[:m, nfull : nfull + 128])

            def build_core(c):
                for slot, (g, e0, e1) in enumerate(cores[c]):
                    build_graph(c, slot, g, e0, e1)

            def dispatch(lo, hi):
                if hi - lo == 1:
                    build_core(lo)
                    return
                mid = (lo + hi) // 2
                with tc.If(pid < mid) as cmp:
                    dispatch(lo, mid)
                with cmp.Else():
                    dispatch(mid, hi)

            dispatch(0, NCORES)

    import os
    if os.environ.get("KERNEL_BUILD_ONLY"):
        return np.zeros((B * MAX_E, HID), np.float32)
    nc.compile()
    if os.environ.get("KERNEL_COMPILE_ONLY"):
        import tempfile
        neff = bass_utils.compile_bass_kernel(nc, tempfile.mkdtemp())
        print("NEFF:", neff)
        return np.zeros((B * MAX_E, HID), np.float32)
    trace = bool(os.environ.get("KERNEL_TRACE"))
    res = bass_utils.run_bass_kernel_spmd(
        nc, in_maps, core_ids=list(range(NCORES)),
        trace=trace,
        trace_cores=list(range(NCORES)) if trace else None,
    )
    global LAST_EXEC_NS, LAST_RESULTS
    LAST_RESULTS = res
    LAST_EXEC_NS = res.exec_time_ns

    out = np.zeros((B * MAX_E, HID), np.float32)
    for c in range(NCORES):
        oc = res.results[c]["out"]
        for slot, (g, e0, e1) in enumerate(cores[c]):
            out[g * MAX_E + e0 : g * MAX_E + e1] = \
                oc[slot * MAX_E + e0 : slot * MAX_E + e1]
    return out


# revision 17
# speedup vs baseline: 3.4243x; 1.1947x over previous
"""Trainium2 Bass kernel for nn_CycleNet_EPD (ragged graph edge-phase decoder).

Math (per graph b, with La = edge_len[b], Ba = beta_len[b]):
  ef[e,:4]   = [x[src_e], x[dst_e]]                        (edge features)
  s[beta,:]  = sum_e |SCB[b,beta,e]| * ef[e,:]             (beta < Ba, e < La)
  emb        = relu(s@W1+b1)@W2+b2                         [Ba,64]
  A[beta,:]  = emb@W3a + b3                                [Ba,128]  (W3a=W3[:64])
  G[e,:]     = ef@W3b                                      [La,128]  (W3b=W3[64:])
  H[e,:]     = sum_{beta<Ba} relu(A[beta,:] + |SCB[b,beta,e]|*G[e,:])
  out[e,:]   = relu((H@W4 + vb)@W5+b5)@W6+b6
               vb = 64*b4 + (64-Ba)*relu(A_pad)@W4  (A_pad: padded-beta row)
  rows with e >= La are zero.

Device mapping (per graph), [h, e] layout, e-chunks of 512:
  - G is rank 4 (G = W3b^T ef), so scb_beta (x) G = W3b^T (ef (x) scb_beta).
    Per 32-beta group one PE "expand" matmul (bf16) broadcasts scb rows to
    4x32 partitions; one DVE multiply with the host-shipped 32x-tiled edge
    features (eft32) builds EFS[4i+k,e] = ef[k,e]*|scb|[g0+i,e] in bf16.
    Per beta, a single K=128 matmul with masked stacked weights w3bm
    (rows 4i..4i+3 = W3b, zeros elsewhere) yields scb_beta*G in PSUM.
  - relu(+A bias) on ACT (majority) / DVE tensor_scalar (minority, balance).
  - The beta-sum is folded into W4: r@W4b accumulates into one PSUM bank
    (start/stop over the chunk's betas); a minority of betas accumulate r on
    DVE into H2 which is flushed through W4b with one extra matmul.
  - out stage: W5/W6 bf16 matmuls with ACT relu/bias epilogues, PE transpose
    to [e, 128], one batched DMA per chunk to DRAM.
  - all hot-loop matmuls bf16 (1 cyc/row); fp32 only in the tiny emb chain.

Sharding: per-core work items (graph, e0, e1); effective load model counts
columns Ba*ne plus per-slice and per-edge-column fixed overheads so cores
with many small graphs are not overloaded. One NEFF; each core's ragged
schedule sits in its own branch of a partition-id If-tree.
Host does only data movement: gather of x rows by edge_index, packing /
replication / dtype casts of inputs, and scatter of per-core outputs into
the full [B*MAX_E, HID] result (padded rows stay zero).
"""

import sys

sys.path.insert(0, "/opt/trn_rl_repo")

import ml_dtypes
import numpy as np

import concourse.bacc as bacc
import concourse.mybir as mybir
import concourse.tile as tile
from concourse import bass_utils

B, MAX_N, MAX_E, MAX_BETA = 16, 512, 1024, 64
NODE_F, HID = 2, 128
NCORES = 8
F32 = mybir.dt.float32
BF16 = mybir.dt.bfloat16
AF = mybir.ActivationFunctionType
ALU = mybir.AluOpType
NPBF16 = ml_dtypes.bfloat16

ECHUNK = 512   # e-tile for stage B / out stage (one PSUM bank)
GFIX = 3000    # planner: per-slice fixed cost (stage A), in column units
OUT_W = 5      # planner: out-stage cost per edge column, in column units


def _relu_on_dve(b):
    return b % 4 == 2


def _acc_on_dve(b):
    return b % 5 in (1, 3)


def _plan(edge_len, beta_len):
    """Per-core work items (g, e0, e1); large graphs split by edge range.

    Effective load = (Ba + OUT_W) * ne + GFIX per slice, balancing stage-B
    columns plus out-stage and per-graph fixed overheads.  Split oversized
    graphs, LPT-assign pieces, then iteratively shave edges from the max
    core onto the min core."""
    La = [max(1, min(MAX_E, int(v))) for v in edge_len]
    Ba = [max(1, min(MAX_BETA, int(v))) for v in beta_len]

    def el(g, ne):
        return (Ba[g] + OUT_W) * ne + GFIX

    total = sum(el(g, La[g]) for g in range(B))
    target = total / NCORES
    pieces = []
    for g in range(B):
        k = max(1, min(round(el(g, La[g]) / target + 0.25), -(-La[g] // 64)))
        base, rem = divmod(La[g], k)
        e0 = 0
        for j in range(k):
            ne = base + (1 if j < rem else 0)
            pieces.append((g, e0, e0 + ne))
            e0 += ne
    pieces.sort(key=lambda p: -el(p[0], p[2] - p[1]))
    cores = [[] for _ in range(NCORES)]
    loads = [0.0] * NCORES
    for p in pieces:
        c = min(range(NCORES), key=lambda i: loads[i])
        cores[c].append(p)
        loads[c] += el(p[0], p[2] - p[1])
    for _ in range(64):  # shave the max core onto the min core
        cM = max(range(NCORES), key=lambda i: loads[i])
        cm = min(range(NCORES), key=lambda i: loads[i])
        surplus = loads[cM] - loads[cm]
        best = None
        for idx, (g, e0, e1) in enumerate(cores[cM]):
            ne_mv = int((surplus / 2 - GFIX) / (Ba[g] + OUT_W))
            ne_mv = min(ne_mv, e1 - e0 - 64)
            if ne_mv >= 64 and (best is None or ne_mv > best[1]):
                best = (idx, ne_mv)
        if best is None:
            break
        idx, ne_mv = best
        g, e0, e1 = cores[cM][idx]
        cores[cM][idx] = (g, e0, e1 - ne_mv)
        cores[cm].append((g, e1 - ne_mv, e1))
        loads[cM] -= (Ba[g] + OUT_W) * ne_mv
        loads[cm] += el(g, ne_mv)
    return La, Ba, cores


def kernel(x, SCB, edge_index, edge_len, beta_len,
           W1, b1, W2, b2, W3, b3, W4, b4, W5, b5, W6, b6):
    x = np.asarray(x, np.float32)
    SCB = np.asarray(SCB, np.float32)
    edge_index = np.asarray(edge_index, np.int32)
    La, Ba, cores = _plan(np.asarray(edge_len), np.asarray(beta_len))
    ngmax = max(len(c) for c in cores)

    # ---- host-side packing (data movement only) ----
    ef_all = []
    for b in range(B):
        src = edge_index[b, 0, : La[b]]
        dst = edge_index[b, 1, : La[b]]
        ef_all.append(np.concatenate([x[b][src], x[b][dst]], axis=1))  # [La,4]

    ef_off = [[0] * ngmax for _ in range(NCORES)]
    emax = 1
    for c in range(NCORES):
        eo = 0
        for i, (g, e0, e1) in enumerate(cores[c]):
            ef_off[c][i] = eo
            eo += La[g]
        emax = max(emax, eo)

    W3b = np.ascontiguousarray(W3[64:], np.float32)       # [4,128]
    exp64 = np.zeros((64, 128), np.float32)               # expand 32b -> 4x32p
    for i in range(64):
        exp64[i, 4 * (i % 32) : 4 * (i % 32) + 4] = 1.0
    w3bm = np.zeros((128, 32 * 128), np.float32)          # masked stacked W3b
    for i in range(32):
        w3bm[4 * i : 4 * i + 4, i * 128 : (i + 1) * 128] = W3b

    # packed constants: one fp32 tensor + one bf16 tensor -> 2 DMAs
    # fp32 [128, 518]: w1(0:64) w2(64:128) w3a(128:256) w4(256:384)
    #   ident(384:512) b1c..b6c,b4x64(512:518)
    CF = 518
    constf = np.zeros((128, CF), np.float32)
    constf[:4, 0:64] = W1
    constf[:64, 64:128] = W2
    constf[:64, 128:256] = W3[:64]
    constf[:, 256:384] = W4
    constf[:, 384:512] = np.eye(128, dtype=np.float32)
    constf[:64, 512] = np.asarray(b1, np.float32)
    constf[:64, 513] = np.asarray(b2, np.float32)
    constf[:, 514] = np.asarray(b3, np.float32)
    constf[:, 515] = 64.0 * np.asarray(b4, np.float32)
    constf[:, 516] = np.asarray(b5, np.float32)
    constf[:, 517] = np.asarray(b6, np.float32)
    # bf16 [128, 4736]: w3bm(0:4096) w4b(4096:4224) w5b(4224:4352)
    #   w6b(4352:4480) exp64b(4480:4608) identb(4608:4736)
    CB = 4736
    constb = np.zeros((128, CB), np.float32)
    constb[:, 0:4096] = w3bm
    constb[:, 4096:4224] = W4
    constb[:, 4224:4352] = W5
    constb[:, 4352:4480] = W6
    constb[:64, 4480:4608] = exp64
    constb[:, 4608:4736] = np.eye(128, dtype=np.float32)
    constb = constb.astype(NPBF16)

    in_maps = []
    for c in range(NCORES):
        scb_pack = np.zeros((64, ngmax * MAX_E), np.float32)
        eft32 = np.zeros((128, emax), np.float32)
        for i, (g, e0, e1) in enumerate(cores[c]):
            scb_pack[:, i * MAX_E : i * MAX_E + La[g]] = SCB[g][:, : La[g]]
            eft32[:, ef_off[c][i] : ef_off[c][i] + La[g]] = \
                np.tile(ef_all[g].T, (32, 1))
        in_maps.append({
            "constf": constf,
            "constb": constb,
            "scb_pack": scb_pack.astype(NPBF16),
            "eft32": eft32.astype(NPBF16),
        })

    # ---- build program ----
    nc = bacc.Bacc("TRN2", target_bir_lowering=False, debug=False,
                   num_devices=NCORES)
    d_in = {}
    for name, arr in in_maps[0].items():
        dt = BF16 if arr.dtype == NPBF16 else F32
        d_in[name] = nc.dram_tensor(name, list(arr.shape), dt,
                                    kind="ExternalInput")
    d_out = nc.dram_tensor("out", [ngmax * MAX_E, HID], F32,
                           kind="ExternalOutput")

    with tile.TileContext(nc) as tc:
        pid = nc.partition_id()
        with (
            tc.tile_pool(name="const", bufs=1) as cpool,
            tc.tile_pool(name="sbA", bufs=2) as sbA,
            tc.tile_pool(name="sbB", bufs=3) as sbB,
            tc.tile_pool(name="psS", bufs=1, space="PSUM") as psS,
            tc.tile_pool(name="psG", bufs=3, space="PSUM") as psG,
            tc.tile_pool(name="psH", bufs=2, space="PSUM") as psH,
            tc.tile_pool(name="psO", bufs=1, space="PSUM") as psO,
        ):
            cf = cpool.tile([128, CF], F32, tag="constf")
            nc.sync.dma_start(cf[:], d_in["constf"].ap())
            cb = cpool.tile([128, CB], BF16, tag="constb")
            nc.sync.dma_start(cb[:], d_in["constb"].ap())
            eftc = cpool.tile([128, emax], BF16, tag="eft32")
            nc.sync.dma_start(eftc[:], d_in["eft32"].ap())
            cst = {
                "w1": cf[:4, 0:64], "w2": cf[:64, 64:128],
                "w3a": cf[:64, 128:256], "w4": cf[:, 256:384],
                "ident": cf[:, 384:512],
                "b1c": cf[:64, 512:513], "b2c": cf[:64, 513:514],
                "b3c": cf[:, 514:515], "b4x64": cf[:, 515:516],
                "b5c": cf[:, 516:517], "b6c": cf[:, 517:518],
                "w3bm": cb[:, 0:4096], "w4b": cb[:, 4096:4224],
                "w5b": cb[:, 4224:4352], "w6b": cb[:, 4352:4480],
                "exp64b": cb[:64, 4480:4608], "identb": cb[:, 4608:4736],
            }

            def build_graph(c, slot, g, es, ee):
                la, ba = La[g], Ba[g]
                nech = (la + 127) // 128  # 128-e chunks for transposes / s
                goff_e = ef_off[c][slot]

                # ---- stage A ----
                scb_sb = sbA.tile([64, MAX_E], BF16, tag="scb")
                nc.sync.dma_start(
                    scb_sb[:, :la],
                    d_in["scb_pack"].ap()[:, slot * MAX_E : slot * MAX_E + la],
                )
                # |SCB| in-place (ACT Abs); rows >= ba never read
                nc.scalar.activation(scb_sb[:ba, :la], scb_sb[:ba, :la],
                                     AF.Abs, bias=0.0, scale=1.0)
                # scb_T chunks [128e, 64b] via PE transpose (for s)
                scbT_sb = sbA.tile([128, 64 * 8], BF16, tag="scbT")
                for ec in range(nech):
                    n = min(128, la - ec * 128)
                    tp = psO.tile([128, 64], BF16, tag="pM")
                    nc.tensor.transpose(
                        tp[:n, :], scb_sb[:, ec * 128 : ec * 128 + n],
                        cst["identb"][:64, :64],
                    )
                    nc.vector.tensor_copy(scbT_sb[:n, ec * 64 : ec * 64 + 64],
                                          tp[:n, :])
                # ef rows [e,4] per 128-chunk (for s)
                efr_sb = sbA.tile([128, 4 * 8], BF16, tag="efr")
                for ec in range(nech):
                    n = min(128, la - ec * 128)
                    tp2 = psO.tile([128, 4], BF16, tag="pM")
                    nc.tensor.transpose(
                        tp2[:n, :],
                        eftc[0:4, goff_e + ec * 128 : goff_e + ec * 128 + n],
                        cst["identb"][:4, :4],
                    )
                    nc.vector.tensor_copy(efr_sb[:n, ec * 4 : ec * 4 + 4],
                                          tp2[:n, :])
                # s_T[k, beta] = sum_e ef[e,k] |scb|_T[e, beta]
                ps_s = psO.tile([4, 128], F32, tag="pM")
                for ec in range(nech):
                    n = min(128, la - ec * 128)
                    nc.tensor.matmul(
                        ps_s[:, :ba],
                        efr_sb[:n, ec * 4 : ec * 4 + 4],
                        scbT_sb[:n, ec * 64 : ec * 64 + ba],
                        start=(ec == 0), stop=(ec == nech - 1),
                    )
                s_sb = sbA.tile([4, 65], F32, tag="s")
                nc.vector.memset(s_sb[:], 0.0)
                nc.vector.tensor_copy(s_sb[:, :ba], ps_s[:, :ba])
                # emb / A chain (one padded col at index ba -> A_pad)
                nb = ba + 1
                pe1 = psO.tile([64, 65], F32, tag="pM")
                nc.tensor.matmul(pe1[:, :nb], cst["w1"], s_sb[:, :nb],
                                 start=True, stop=True)
                e1_sb = sbA.tile([64, 65], F32, tag="e1")
                nc.scalar.activation(e1_sb[:, :nb], pe1[:, :nb], AF.Relu,
                                     bias=cst["b1c"], scale=1.0)
                pe2 = psO.tile([64, 65], F32, tag="pM")
                nc.tensor.matmul(pe2[:, :nb], cst["w2"], e1_sb[:, :nb],
                                 start=True, stop=True)
                e2_sb = sbA.tile([64, 65], F32, tag="e2")
                nc.scalar.activation(e2_sb[:, :nb], pe2[:, :nb], AF.Identity,
                                     bias=cst["b2c"], scale=1.0)
                pa = psO.tile([128, 65], F32, tag="pM")
                nc.tensor.matmul(pa[:, :nb], cst["w3a"], e2_sb[:, :nb],
                                 start=True, stop=True)
                A_sb = sbA.tile([128, 65], F32, tag="A")
                nc.scalar.activation(A_sb[:, :nb], pa[:, :nb], AF.Identity,
                                     bias=cst["b3c"], scale=1.0)
                # K0 = relu(A_pad); vb = (64-Ba) * K0@W4 + 64*b4
                K0_sb = sbA.tile([128, 1], F32, tag="K0")
                nc.scalar.activation(K0_sb[:], A_sb[:, ba : ba + 1], AF.Relu,
                                     bias=0.0, scale=1.0)
                pk = psO.tile([128, 1], F32, tag="pM")
                nc.tensor.matmul(pk[:], cst["w4"], K0_sb[:],
                                 start=True, stop=True)
                vb_sb = sbA.tile([128, 1], F32, tag="vb")
                nc.scalar.activation(vb_sb[:], pk[:], AF.Identity,
                                     bias=cst["b4x64"],
                                     scale=float(64 - ba))

                # ---- stage B + out stage, per 512-e chunk ----
                pe_acc = [b for b in range(ba) if not _acc_on_dve(b)]
                dve_acc = [b for b in range(ba) if _acc_on_dve(b)]
                if not pe_acc:  # guarantee pW4 gets a start
                    pe_acc = [dve_acc.pop(0)]
                for e0 in range(es, ee, ECHUNK):
                    n = min(ECHUNK, ee - e0)
                    ecol = goff_e + e0  # column of this chunk in eft32
                    pW4 = psH.tile([128, ECHUNK], F32, tag="H")
                    H2 = sbB.tile([128, ECHUNK], F32, tag="H2")
                    h2_open = False
                    for g0 in range(0, ba, 32):
                        gsz = min(32, ba - g0)
                        P1 = psS.tile([128, ECHUNK], F32, tag="P1")
                        nc.tensor.matmul(
                            P1[:, :n],
                            cst["exp64b"][g0 : g0 + gsz, :],
                            scb_sb[g0 : g0 + gsz, e0 : e0 + n],
                            start=True, stop=True)
                        efs = sbB.tile([128, ECHUNK], BF16, tag="efs")
                        nc.vector.tensor_mul(efs[:, :n], P1[:, :n],
                                             eftc[:, ecol : ecol + n])
                        for i in range(gsz):
                            b = g0 + i
                            pG = psG.tile([128, ECHUNK], F32, tag="pG")
                            nc.tensor.matmul(
                                pG[:, :n],
                                cst["w3bm"][:, i * 128 : (i + 1) * 128],
                                efs[:, :n], start=True, stop=True)
                            r = sbB.tile([128, ECHUNK], BF16, tag="r")
                            if _relu_on_dve(b):
                                nc.vector.tensor_scalar(
                                    r[:, :n], pG[:, :n],
                                    A_sb[:, b : b + 1], 0.0,
                                    ALU.add, ALU.max)
                            else:
                                nc.scalar.activation(
                                    r[:, :n], pG[:, :n], AF.Relu,
                                    bias=A_sb[:, b : b + 1], scale=1.0)
                            if b in pe_acc:
                                nc.tensor.matmul(
                                    pW4[:, :n], cst["w4b"], r[:, :n],
                                    start=(b == pe_acc[0]),
                                    stop=(b == pe_acc[-1] and not dve_acc))
                            elif not h2_open:
                                nc.vector.tensor_copy(H2[:, :n], r[:, :n])
                                h2_open = True
                            else:
                                nc.vector.tensor_add(H2[:, :n], H2[:, :n],
                                                     r[:, :n])
                    if dve_acc:
                        H2b = sbB.tile([128, ECHUNK], BF16, tag="H2b")
                        nc.vector.tensor_copy(H2b[:, :n], H2[:, :n])
                        nc.tensor.matmul(pW4[:, :n], cst["w4b"],
                                         H2b[:, :n], start=False, stop=True)
                    # out stage: h = pW4 + vb; relu(h@W5+b5)@W6+b6
                    r5 = sbB.tile([128, ECHUNK], BF16, tag="r5")
                    nc.scalar.activation(r5[:, :n], pW4[:, :n], AF.Identity,
                                         bias=vb_sb[:], scale=1.0)
                    p2 = psO.tile([128, ECHUNK], F32, tag="pO")
                    nc.tensor.matmul(p2[:, :n], cst["w5b"], r5[:, :n],
                                     start=True, stop=True)
                    r6 = sbB.tile([128, ECHUNK], BF16, tag="r6")
                    nc.scalar.activation(r6[:, :n], p2[:, :n], AF.Relu,
                                         bias=cst["b5c"], scale=1.0)
                    p3 = psO.tile([128, ECHUNK], F32, tag="pO")
                    nc.tensor.matmul(p3[:, :n], cst["w6b"], r6[:, :n],
                                     start=True, stop=True)
                    o_sb = sbB.tile([128, ECHUNK], F32, tag="o")
                    nc.scalar.activation(o_sb[:, :n], p3[:, :n], AF.Identity,
                                         bias=cst["b6c"], scale=1.0)
                    oT = sbB.tile([128, ECHUNK], F32, tag="oT")
                    for et in range(0, n, 128):
                        m = min(128, n - et)
                        po = psO.tile([128, 128], F32, tag="pM")
                        nc.tensor.transpose(po[:m, :], o_sb[:, et : et + m],
                                            cst["ident"])
                        nc.vector.tensor_copy(oT[:m, et : et + 128], po[:m, :])
                    r0 = slot * MAX_E + e0
                    nfull = (n // 128) * 128
                    if nfull:
                        dst = d_out.ap()[r0 : r0 + nfull, :].rearrange(
                            "(et p) h -> p et h", p=128)
                        nc.sync.dma_start(dst, oT[:, :nfull])
                    if n > nfull:
                        m = n - nfull
                        nc.sync.dma_start(
                            d_out.ap()[r0 + nfull : r0 + n, :],
                            oT[:m, nfull : nfull + 128])

            def build_core(c):
                for slot, (g, e0, e1) in enumerate(cores[c]):
                    build_graph(c, slot, g, e0, e1)

            def dispatch(lo, hi):
                if hi - lo == 1:
                    build_core(lo)
                    return
                mid = (lo + hi) // 2
                with tc.If(pid < mid) as cmp:
                    dispatch(lo, mid)
                with cmp.Else():
                    dispatch(mid, hi)

            dispatch(0, NCORES)

    import os
    if os.environ.get("KERNEL_BUILD_ONLY"):
        return np.zeros((B * MAX_E, HID), np.float32)
    nc.compile()
    if os.environ.get("KERNEL_COMPILE_ONLY"):
        import tempfile
        neff = bass_utils.compile_bass_kernel(nc, tempfile.mkdtemp())
        print("NEFF:", neff)
        return np.zeros((B * MAX_E, HID), np.float32)
    trace = bool(os.environ.get("KERNEL_TRACE"))
    res = bass_utils.run_bass_kernel_spmd(
        nc, in_maps, core_ids=list(range(NCORES)),
        trace=trace,
        trace_cores=list(range(NCORES)) if trace else None,
    )
    global LAST_EXEC_NS, LAST_RESULTS
    LAST_RESULTS = res
    LAST_EXEC_NS = res.exec_time_ns

    out = np.zeros((B * MAX_E, HID), np.float32)
    for c in range(NCORES):
        oc = res.results[c]["out"]
        for slot, (g, e0, e1) in enumerate(cores[c]):
            out[g * MAX_E + e0 : g * MAX_E + e1] = \
                oc[slot * MAX_E + e0 : slot * MAX_E + e1]
    return out
